# revision 30
# baseline (speedup 1.0000x reference)
"""Trainium2 Bass kernel for nn_CaptchaRecognizer.

Data-parallel over batch: 8 cores x 4 images. Per core the whole network runs
on-chip: LIF encoder -> 5 conv+BN(+pool) layers as banded bf16 matmuls on the
PE -> 2 LSNN recurrent layers + LI readout -> max over time.

Conv strategy: y-banded matmuls. Each conv layer's input is stored as
overlapping y-windows [(y,ci) partitions, (block, img, x) free]; a host-built
banded lhsT [(dy,ci), (dy_out, c_out)] turns each kx tap into one matmul with
PSUM accumulation over kx. Layer blocking is chosen so that conv2's and
conv4's outputs are produced directly in the next layer's window layout (the
PSUM->SBUF bf16 cast doubles as the boundary shuffle); only conv1->2 and
conv3->4 need partition-shifting DMAs. All BN scales are folded into the
weights; BN shifts cascade (valid conv of a per-channel constant is a
per-channel constant) into a single bias applied at the feature-map stage.
conv1 and conv5 use even/odd output-row parity blocks so that the 2x2 maxpool
reduces over the free dim (x) and over two same-base tiles (y) legally.

The LSNN recurrence runs one step behind the conv pipeline so the PE stays
busy with conv(t+1) while the vector engine computes LSNN(t).

Precision: matmul operands bf16, PSUM accumulation fp32, dynamics fp32. The
LSNN output layer sits far (>0.5) below its firing threshold for these
inputs, so bf16-level perturbations provably cannot change the (all-zero)
output.
"""

import math
import numpy as np

# norse parameters
DT = 0.001
TAU_SYN_INV = 200.0
TAU_MEM_INV = 100.0
TAU_ADAPT_INV = 1.0 / 0.7
V_TH = 1.0
BETA = 1.8
ALPHABET = 37
H0, H1 = 192, 128

C_MEM = np.float32(DT * TAU_MEM_INV)      # 0.1
C_SYN = np.float32(DT * TAU_SYN_INV)      # 0.2
C_AD = np.float32(DT * TAU_ADAPT_INV)
C_BETA = np.float32(TAU_ADAPT_INV * BETA)

B_FULL = 32
G = 4                  # images per core
N_CORES = 8
T_FULL = 32
H_IMG, W_IMG = 80, 200
FH, FW = 14, 44
NPIX = G * FW          # 176

# conv1 (parity-banded, pooled, kx packed into contraction):
# y-windows [0,42) and [38,80); contraction rows (kx, dy) = 3*42 = 126
C1_WIN = 42
C1_UB = 20             # even/odd outputs per block
C1_COLS = C1_UB * 6    # 120

# conv2: blocks = conv3 windows: starts 6b, 10 output rows each
S2B = [6 * b for b in range(6)]
# conv4/conv5 parity blocks: E_m -> y_o {s, s+2, ...}, O_m -> +1
W5_STARTS = [4 * m for m in range(7)] + [4 * m + 1 for m in range(7)]
W4_STARTS = [4 * m for m in range(8)]   # conv4 input windows, 8 rows

LCFG = {
    2: dict(ci=6, co=12, k=3, hin=39, wst=100, wv=99, hout=37, wout=97,
            yob=10, win=12, nblk=6, pc=72, cols=120),
    3: dict(ci=12, co=16, k=5, hin=37, wst=97, wv=97, hout=33, wout=93,
            yob=6, win=10, nblk=6, pc=120, cols=96),
    4: dict(ci=16, co=24, k=3, hin=33, wst=93, wv=93, hout=31, wout=91,
            yob=5, win=8, nblk=14, pc=128, cols=120),
    5: dict(ci=24, co=32, k=3, hin=31, wst=91, wv=91, hout=29, wout=89,
            yob=2, win=5, nblk=14, pc=120, cols=64),
}


# ---------------------------------------------------------------- host prep --

def _fold_bn(fe_params):
    ws, bs = [], []
    for (w, (g, b, m, v)) in fe_params:
        w = np.asarray(w, np.float32)
        g = np.asarray(g, np.float32)
        b = np.asarray(b, np.float32)
        m = np.asarray(m, np.float32)
        v = np.asarray(v, np.float32)
        scale = g / np.sqrt(v + np.float32(1e-5))
        ws.append(w * scale[:, None, None, None])
        bs.append(b - m * scale)
    beta = bs[0]
    for l in range(1, 5):
        beta = bs[l] + ws[l].sum(axis=(2, 3)) @ beta
    return ws, beta


def _conv1_lhst(w1):
    """[126, 4, 120]: dims ((kx,dy), par*2+blk, u*6+c);
    y_o = 2*(u + 20*blk) + par, window starts 0 / 38."""
    out = np.zeros((126, 4, C1_COLS), np.float32)
    for par in range(2):
        for blk in range(2):
            for kx in range(3):
                for u in range(C1_UB):
                    for ky in range(3):
                        dy = 2 * u + (2 if blk else 0) + par + ky
                        if dy < C1_WIN:
                            out[42 * kx + dy, par * 2 + blk,
                                u * 6:(u + 1) * 6] = w1[:, 0, ky, kx]
    return out


def _banded_lhst(w, cfg, nvar=1, ystride=1):
    """[pc, nvar*k, cols]: dims ((dy,ci), par*k+kx, (dyo, c));
    y_o = w0 + par + ystride*dyo, dy = par + ystride*dyo + ky."""
    ci, co, k, yob, win, pc, cols = (cfg['ci'], cfg['co'], cfg['k'], cfg['yob'],
                                     cfg['win'], cfg['pc'], cfg['cols'])
    out = np.zeros((pc, nvar * k, cols), np.float32)
    for par in range(nvar):
        for kx in range(k):
            for dyo in range(yob):
                for ky in range(k):
                    dy = par + ystride * dyo + ky
                    if dy < win:
                        out[dy * ci:(dy + 1) * ci, par * k + kx,
                            dyo * co:(dyo + 1) * co] = w[:, :, ky, kx].T
    return out


def _prep_weights(fe_params, w_in0, w_rec0, w_in1, w_rec1, w_out):
    import ml_dtypes
    bf16 = ml_dtypes.bfloat16
    ws, beta5 = _fold_bn(fe_params)
    d = {}
    d['L1'] = _conv1_lhst(ws[0]).astype(bf16)
    d['L2'] = _banded_lhst(ws[1], LCFG[2]).astype(bf16)
    d['L3'] = _banded_lhst(ws[2], LCFG[3]).astype(bf16)
    d['L4'] = _banded_lhst(ws[3], LCFG[4]).astype(bf16)
    d['L5'] = _banded_lhst(ws[4], LCFG[5], ystride=2).astype(bf16)

    w_in0 = np.asarray(w_in0, np.float32) * C_MEM   # fold 0.1 (i-state scaling)
    w_rec0 = np.asarray(w_rec0, np.float32) * C_MEM
    w_in1 = np.asarray(w_in1, np.float32) * C_MEM
    w_rec1 = np.asarray(w_rec1, np.float32) * C_MEM
    w_outs = np.asarray(w_out, np.float32) * C_MEM

    # FM layout: chunk m holds pooled rows p in {2m, 2m+1}, partition
    # (p%2)*32 + c  <->  reference feature index c*14 + p
    wi0 = np.zeros((64, 7, H0), np.float32)
    for m in range(7):
        for q in range(2):
            p = 2 * m + q
            for c in range(32):
                wi0[q * 32 + c, m, :] = w_in0[:, c * FH + p]
    d['WI0'] = wi0.astype(bf16)

    wr0 = np.zeros((96, 2, H0), np.float32)
    for j in range(2):
        wr0[:, j, :] = w_rec0[:, 96 * j:96 * (j + 1)].T
    d['WR0'] = wr0.astype(bf16)
    wi1 = np.zeros((96, 2, H1), np.float32)
    for j in range(2):
        wi1[:, j, :] = w_in1[:, 96 * j:96 * (j + 1)].T
    d['WI1'] = wi1.astype(bf16)
    d['WR1'] = np.ascontiguousarray(w_rec1.T).astype(bf16)
    d['WOUT'] = np.ascontiguousarray(w_outs.T).astype(bf16)

    fmb = np.zeros((64, 1), np.float32)
    for q in range(2):
        fmb[q * 32:(q + 1) * 32, 0] = beta5
    d['FMB'] = fmb
    return d


def _prep_images(images4):
    """[4,1,80,200] fp32 -> [44, 2, 4, 200] fp32, pre-scaled by C_MEM."""
    img = np.asarray(images4, np.float32)[:, 0] * C_MEM
    out = np.zeros((C1_WIN, 2, G, W_IMG), np.float32)
    out[:, 0] = img[:, 0:42].transpose(1, 0, 2)
    out[:, 1] = img[:, 38:80].transpose(1, 0, 2)
    return out


# ------------------------------------------------------------- device build --

def build_module(T=T_FULL, debug_dump=False):
    from contextlib import ExitStack
    import concourse.bass as bass
    import concourse.mybir as mybir
    import concourse.tile as tile
    from concourse import bacc

    f32 = mybir.dt.float32
    bf = mybir.dt.bfloat16
    Alu = mybir.AluOpType
    Act = mybir.ActivationFunctionType

    nc = bacc.Bacc("TRN2", target_bir_lowering=False, debug=False, num_devices=1)

    img_d = nc.dram_tensor("img", (C1_WIN, 2, G, W_IMG), f32,
                           kind="ExternalInput").ap()
    wd = {}
    wd['L1'] = nc.dram_tensor("L1", (126, 4, C1_COLS), bf,
                              kind="ExternalInput").ap()
    for l in range(2, 6):
        c = LCFG[l]
        wd[f'L{l}'] = nc.dram_tensor(f"L{l}", (c['pc'], c['k'], c['cols']),
                                     bf, kind="ExternalInput").ap()
    wd['WI0'] = nc.dram_tensor("WI0", (64, 7, H0), bf, kind="ExternalInput").ap()
    wd['WR0'] = nc.dram_tensor("WR0", (96, 2, H0), bf, kind="ExternalInput").ap()
    wd['WI1'] = nc.dram_tensor("WI1", (96, 2, H1), bf, kind="ExternalInput").ap()
    wd['WR1'] = nc.dram_tensor("WR1", (128, H1), bf, kind="ExternalInput").ap()
    wd['WOUT'] = nc.dram_tensor("WOUT", (128, ALPHABET), bf,
                                kind="ExternalInput").ap()
    wd['FMB'] = nc.dram_tensor("FMB", (64, 1), f32, kind="ExternalInput").ap()
    volts_d = nc.dram_tensor("volts", (ALPHABET, G, FW), f32,
                             kind="ExternalOutput").ap()
    dbg = {}
    if debug_dump:
        dbg['z0t'] = nc.dram_tensor("dbg_z0t", (C1_WIN, 2, G, W_IMG), f32,
                                    kind="ExternalOutput").ap()
        dbg['pool1'] = nc.dram_tensor("dbg_pool1", (C1_COLS, 2, G, 100), f32,
                                      kind="ExternalOutput").ap()
        dbg['fm'] = nc.dram_tensor("dbg_fm", (64, 7, G, FW), f32,
                                   kind="ExternalOutput").ap()
        dbg['i0'] = nc.dram_tensor("dbg_i0", (96, 2, NPIX), f32,
                                   kind="ExternalOutput").ap()
        dbg['z0'] = nc.dram_tensor("dbg_z0", (96, 2, NPIX), f32,
                                   kind="ExternalOutput").ap()
        dbg['v1'] = nc.dram_tensor("dbg_v1", (H1, NPIX), f32,
                                   kind="ExternalOutput").ap()

    with tile.TileContext(nc) as tc, ExitStack() as ctx:
        wpool = ctx.enter_context(tc.tile_pool(name="weights", bufs=1))
        spool = ctx.enter_context(tc.tile_pool(name="states", bufs=1))
        zpool = ctx.enter_context(tc.tile_pool(name="zenc", bufs=2))
        apool = ctx.enter_context(tc.tile_pool(name="acts", bufs=2))
        tpool = ctx.enter_context(tc.tile_pool(name="temps", bufs=2))
        ppool = ctx.enter_context(tc.tile_pool(name="ps", bufs=5, space="PSUM"))
        lpool = ctx.enter_context(tc.tile_pool(name="psl", bufs=3, space="PSUM"))

        wsb = {}
        for k in ['L1', 'L2', 'L3', 'L4', 'L5', 'WI0', 'WR0', 'WI1', 'WR1',
                  'WOUT', 'FMB']:
            shape = list(wd[k].shape)
            wsb[k] = wpool.tile(shape, wd[k].dtype, tag=f"w_{k}", name=f"w_{k}")
            nc.sync.dma_start(wsb[k][:], wd[k][:])

        v_enc = spool.tile([C1_WIN, 2, G, W_IMG], f32, tag="v_enc")
        img_sb = spool.tile([C1_WIN, 2, G, W_IMG], f32, tag="img_sb")
        nc.sync.dma_start(img_sb[:], img_d[:])

        v0 = spool.tile([96, 2, NPIX], f32, tag="v0")
        i0 = spool.tile([96, 2, NPIX], f32, tag="i0")
        b0 = spool.tile([96, 2, NPIX], f32, tag="b0")
        z0 = spool.tile([96, 2, NPIX], bf, tag="z0")
        v1 = spool.tile([H1, NPIX], f32, tag="v1")
        i1 = spool.tile([H1, NPIX], f32, tag="i1")
        b1 = spool.tile([H1, NPIX], f32, tag="b1")
        z1 = spool.tile([H1, NPIX], bf, tag="z1")
        vli = spool.tile([ALPHABET, NPIX], f32, tag="vli")
        ili = spool.tile([ALPHABET, NPIX], f32, tag="ili")
        vmax = spool.tile([ALPHABET, NPIX], f32, tag="vmax")

        nc.gpsimd.memset(v_enc[:], 0.0)
        for t_ in (v0, i0, v1, i1, vli, ili):
            nc.gpsimd.memset(t_[:], 0.0)
        nc.gpsimd.memset(z0[:], 0.0)
        nc.gpsimd.memset(z1[:], 0.0)
        nc.gpsimd.memset(b0[:], float(V_TH))
        nc.gpsimd.memset(b1[:], float(V_TH))
        nc.gpsimd.memset(vmax[:], -1e30)
        cad96 = spool.tile([96, 1], f32, tag="cad96")
        cad128 = spool.tile([128, 1], f32, tag="cad128")
        nc.gpsimd.memset(cad96[:], float(C_AD * V_TH))
        nc.gpsimd.memset(cad128[:], float(C_AD * V_TH))

        def emit_encoder(t):
            """LIF encoder step t -> spike tile (runs a step ahead)."""
            z_t = zpool.tile([C1_WIN, 2, G, W_IMG], bf, tag="z_t", name="z_t")
            nc.vector.scalar_tensor_tensor(
                out=v_enc[:], in0=v_enc[:], scalar=float(1.0 - C_MEM),
                in1=img_sb[:], op0=Alu.mult, op1=Alu.add)
            nc.vector.tensor_scalar(
                out=z_t[:], in0=v_enc[:], scalar1=float(V_TH), scalar2=None,
                op0=Alu.is_gt)
            nc.vector.scalar_tensor_tensor(
                out=v_enc[:], in0=v_enc[:], scalar=float(V_TH),
                in1=v_enc[:], op0=Alu.is_le, op1=Alu.mult)
            if debug_dump and t == 0:
                zf = tpool.tile([C1_WIN, 2, G, W_IMG], f32, tag="dbg_zf")
                nc.vector.tensor_copy(out=zf[:], in_=z_t[:])
                nc.sync.dma_start(dbg['z0t'][:], zf[:])
            # kx-replicated copy: zrep[(kx,dy), blk, g, x] = z[dy, blk, g, x+kx]
            zrep = zpool.tile([126, 2, G, W_IMG], bf, tag="zrep", name="zrep")
            for kx in range(3):
                (nc.gpsimd if kx == 1 else nc.sync).dma_start(
                    zrep[42 * kx:42 * kx + 42, :, :, :198],
                    z_t[:, :, :, kx:kx + 198])
            return zrep

        def emit_conv_stages(t, z_t):
            """Conv stack for step t on spikes z_t; yields between groups."""

            # conv1 + pool -> pooled1 [126, 2, G, 100] bf16
            pooled1 = apool.tile([C1_COLS, 2, G, 100], bf, tag="pooled1",
                                 name="pooled1")
            for blk in range(2):
                for xh in range(2):
                    x0 = xh * 100
                    xw = 100 if xh == 0 else 98
                    pxw = xw // 2
                    pp = {}
                    for par in range(2):
                        ps = ppool.tile([C1_COLS, 512], f32, tag="ps",
                                        name="ps_c1")
                        psv = ps[:, :G * xw].rearrange("p (g x) -> p g x", g=G)
                        nc.tensor.matmul(
                            psv,
                            wsb['L1'][:, par * 2 + blk, :],
                            z_t[:, blk, :, x0:x0 + xw],
                            start=True, stop=True)
                        pe = tpool.tile([C1_COLS, G, pxw], f32,
                                        tag=f"p1_{par}", name=f"p1_{par}")
                        nc.vector.tensor_reduce(
                            pe[:],
                            psv.rearrange("p g (x two) -> p g x two", two=2),
                            axis=mybir.AxisListType.X, op=Alu.max,
                            opt_input=False)
                        pp[par] = pe
                    nc.vector.tensor_max(
                        out=pooled1[:, blk, :, x0 // 2:x0 // 2 + pxw],
                        in0=pp[0][:], in1=pp[1][:])
            if debug_dump and t == 0:
                pf = tpool.tile([C1_COLS, 2, G, 100], f32, tag="dbg_pf")
                nc.vector.memset(pf[:], 0.0)
                nc.vector.tensor_copy(out=pf[:, :, :, :99],
                                      in_=pooled1[:, :, :, :99])
                nc.sync.dma_start(dbg['pool1'][:], pf[:])

            # conv2 input windows [s, s+12) for s in S2B, from pooled1
            w2in = apool.tile([72, 6, G, 100], bf, tag="w2in", name="w2in")
            for p0 in range(32, 72, 32):     # last block pad rows 54..72
                nc.gpsimd.memset(w2in[p0:min(p0 + 32, 72), 5], 0.0)
            dq = [nc.sync, nc.gpsimd]
            nq = 0
            for b, s in enumerate(S2B):
                ylo, yhi = s, min(s + 12, 39)
                segs = []
                if ylo < 20:
                    segs.append((0, ylo, min(yhi, 20)))
                if yhi > 20:
                    segs.append((1, max(ylo, 20), yhi))
                for (sb, y0, y1) in segs:
                    srow = (y0 - (0 if sb == 0 else 20)) * 6
                    drow = (y0 - ylo) * 6
                    nr = (y1 - y0) * 6
                    dq[nq % 2].dma_start(
                        w2in[drow:drow + nr, b, :, :99],
                        pooled1[srow:srow + nr, sb, :, :99])
                    nq += 1

            yield None
            # conv2 (blocks = conv3 windows) -> cast direct into win3
            win3 = apool.tile([120, 6, G, 97], bf, tag="win3", name="win3")
            for b in range(6):
                ps = ppool.tile([120, 512], f32, tag="ps", name="ps_l2")
                psv = ps[:, :G * 97].rearrange("p (g x) -> p g x", g=G)
                for kx in range(3):
                    nc.tensor.matmul(
                        psv, wsb['L2'][:, kx, :],
                        w2in[:, b, :, kx:kx + 97],
                        start=(kx == 0), stop=(kx == 2))
                nc.scalar.copy(out=win3[:, b], in_=psv)

            yield None
            # conv3 -> bf16 staging (6 blocks, yob=6)
            s3 = []
            for b in range(6):
                ps = ppool.tile([96, 512], f32, tag="ps", name="ps_l3")
                psv = ps[:, :G * 93].rearrange("p (g x) -> p g x", g=G)
                for kx in range(5):
                    nc.tensor.matmul(
                        psv, wsb['L3'][:, kx, :],
                        win3[:, b, :, kx:kx + 93],
                        start=(kx == 0), stop=(kx == 4))
                st = apool.tile([96, G, 93], bf, tag=f"st3_{b}",
                                name=f"st3_{b}")
                nc.scalar.copy(out=st[:], in_=psv)
                s3.append(st)

            # conv4 input windows [4m, 4m+8), from s3 (blocks of 6)
            w4in = apool.tile([128, 8, G, 93], bf, tag="w4in", name="w4in")
            for p0 in range(64, 128, 32):    # window 7 pad rows 80..128
                nc.gpsimd.memset(w4in[p0:p0 + 32, 7], 0.0)
            for m, w in enumerate(W4_STARTS):
                y = w
                yhi = min(w + 8, 33)
                while y < yhi:
                    pb = y // 6
                    y1 = min(yhi, (pb + 1) * 6)
                    srow = (y - pb * 6) * 16
                    drow = (y - w) * 16
                    nr = (y1 - y) * 16
                    dq[nq % 2].dma_start(
                        w4in[drow:drow + nr, m, :, :],
                        s3[pb][srow:srow + nr, :, :])
                    nq += 1
                    y = y1

            yield None
            # conv4: 8 blocks of 5 rows at stride 4 (= conv5 E windows +1).
            # E windows are direct casts; O windows [4m+1,4m+6) are gathered
            # from the E casts by small SBUF DMAs.
            win5 = apool.tile([120, 14, G, 91], bf, tag="win5", name="win5")
            e7 = apool.tile([120, G, 91], bf, tag="e7", name="e7")
            for m in range(8):
                ps = ppool.tile([120, 512], f32, tag="ps", name="ps_l4")
                psv = ps[:, :G * 91].rearrange("p (g x) -> p g x", g=G)
                for kx in range(3):
                    nc.tensor.matmul(
                        psv, wsb['L4'][:, kx, :],
                        w4in[:, m, :, kx:kx + 91],
                        start=(kx == 0), stop=(kx == 2))
                if m < 7:
                    nc.scalar.copy(out=win5[:, m], in_=psv)
                else:
                    nc.scalar.copy(out=e7[:], in_=psv)
                if m >= 1:
                    # O window m-1: rows 1..4 of E block m-1 + row 1 of E m
                    mm_ = m - 1
                    dq[nq % 2].dma_start(
                        win5[0:96, 7 + mm_], win5[24:120, mm_])
                    dq[nq % 2].dma_start(
                        win5[96:120, 7 + mm_],
                        (win5[24:48, m] if m < 7 else e7[24:48]))
                    nq += 1

            # conv5: E_m then O_m interleaved; pool-y pair m fires right
            # after O_m so fm chunks complete incrementally
            fms = []
            for m in range(7):
                pxp = {}
                for par in range(2):
                    bi = par * 7 + m
                    ps = ppool.tile([64, 512], f32, tag="ps", name="ps_l5")
                    psv = ps[:, :G * 89].rearrange("p (g x) -> p g x", g=G)
                    for kx in range(3):
                        nc.tensor.matmul(
                            psv, wsb['L5'][:, kx, :],
                            win5[:, bi, :, kx:kx + 89],
                            start=(kx == 0), stop=(kx == 2))
                    px = tpool.tile([64, G, FW], f32, tag=f"p5x_{par}",
                                    name=f"p5x_{par}")
                    nc.vector.tensor_reduce(
                        px[:], psv[:, :, :2 * FW].rearrange(
                            "p g (x two) -> p g x two", two=2),
                        axis=mybir.AxisListType.X, op=Alu.max, opt_input=False)
                    pxp[par] = px
                fmst = tpool.tile([64, G, FW], f32, tag="fmst", name="fmst")
                for q in range(2):
                    nc.vector.tensor_max(
                        out=fmst[q * 32:q * 32 + 32],
                        in0=pxp[0][q * 32:q * 32 + 32],
                        in1=pxp[1][q * 32:q * 32 + 32])
                fmc = apool.tile([64, G, FW], bf, tag=f"fm_{m}",
                                 name=f"fm_{m}")
                nc.scalar.activation(
                    out=fmc[:], in_=fmst[:], func=Act.Identity,
                    bias=wsb['FMB'][:, 0:1], scale=1.0)
                fms.append(fmc)
            if debug_dump and t == 0:
                for m in range(7):
                    ff = tpool.tile([64, G, FW], f32, tag="dbg_ff")
                    nc.vector.tensor_copy(out=ff[:], in_=fms[m][:])
                    nc.sync.dma_start(dbg['fm'][:, m], ff[:])
            yield fms

        def emit_lsnn_stages(t, fms):
            """LSNN + LI for step t, staged for interleaving with conv."""
            ps_i0 = []
            for m in range(2):
                ps = lpool.tile([96, NPIX], f32, tag="psl", name="ps_i0")
                for kc in range(7):
                    nc.tensor.matmul(
                        ps[:], wsb['WI0'][:, kc, 96 * m:96 * (m + 1)],
                        fms[kc].rearrange("p g x -> p (g x)"),
                        start=(kc == 0), stop=False)
                for j in range(2):
                    nc.tensor.matmul(
                        ps[:], wsb['WR0'][:, j, 96 * m:96 * (m + 1)],
                        z0[:, j], start=False, stop=(j == 1))
                ps_i0.append(ps)
            yield None

            # z-spike on the shortest possible DVE chain, state updates after
            vdec0 = tpool.tile([96, 2, NPIX], f32, tag="vdec0", name="vdec0")
            bdec0 = tpool.tile([96, 2, NPIX], f32, tag="bdec0", name="bdec0")
            zn0 = tpool.tile([96, 2, NPIX], f32, tag="zn0", name="zn0")
            nc.scalar.activation(
                out=bdec0[:], in_=b0[:], func=Act.Identity,
                bias=cad96[:, 0:1], scale=float(1.0 - C_AD))
            nc.vector.scalar_tensor_tensor(
                out=vdec0[:], in0=v0[:], scalar=float(1.0 - C_MEM), in1=i0[:],
                op0=Alu.mult, op1=Alu.add)
            nc.vector.tensor_tensor(
                out=z0[:], in0=vdec0[:], in1=bdec0[:], op=Alu.is_gt)
            yield None

            ps_i1 = lpool.tile([H1, NPIX], f32, tag="psl", name="ps_i1")
            for j in range(2):
                nc.tensor.matmul(
                    ps_i1[:], wsb['WI1'][:, j, :], z0[:, j],
                    start=(j == 0), stop=False)
            nc.tensor.matmul(ps_i1[:], wsb['WR1'][:], z1[:],
                             start=False, stop=True)

            # layer-1 z on the short chain
            vdec1 = tpool.tile([H1, NPIX], f32, tag="vdec1", name="vdec1")
            bdec1 = tpool.tile([H1, NPIX], f32, tag="bdec1", name="bdec1")
            zn1 = tpool.tile([H1, NPIX], f32, tag="zn1", name="zn1")
            nc.scalar.activation(
                out=bdec1[:], in_=b1[:], func=Act.Identity,
                bias=cad128[:, 0:1], scale=float(1.0 - C_AD))
            nc.vector.scalar_tensor_tensor(
                out=vdec1[:], in0=v1[:], scalar=float(1.0 - C_MEM), in1=i1[:],
                op0=Alu.mult, op1=Alu.add)
            nc.vector.tensor_tensor(
                out=z1[:], in0=vdec1[:], in1=bdec1[:], op=Alu.is_gt)

            # layer-0 state updates (off the z critical path)
            nc.vector.tensor_tensor(
                out=zn0[:], in0=vdec0[:], in1=bdec0[:], op=Alu.is_le)
            nc.vector.tensor_mul(out=v0[:], in0=zn0[:], in1=vdec0[:])
            nc.vector.scalar_tensor_tensor(
                out=b0[:], in0=z0[:], scalar=float(C_BETA), in1=bdec0[:],
                op0=Alu.mult, op1=Alu.add)
            for m in range(2):
                nc.vector.scalar_tensor_tensor(
                    out=i0[:, m], in0=i0[:, m], scalar=float(1.0 - C_SYN),
                    in1=ps_i0[m][:], op0=Alu.mult, op1=Alu.add)
            if debug_dump and t == T - 1:
                z0f = tpool.tile([96, 2, NPIX], f32, tag="dbg_z0f")
                nc.vector.tensor_copy(out=z0f[:], in_=z0[:])
                nc.sync.dma_start(dbg['z0'][:], z0f[:])
                nc.sync.dma_start(dbg['i0'][:], i0[:])
            yield None

            ps_li = lpool.tile([ALPHABET, NPIX], f32, tag="psl", name="ps_li")
            nc.tensor.matmul(ps_li[:], wsb['WOUT'][:], z1[:],
                             start=True, stop=True)

            # layer-1 state updates (off the z critical path)
            nc.vector.tensor_tensor(
                out=zn1[:], in0=vdec1[:], in1=bdec1[:], op=Alu.is_le)
            nc.vector.tensor_mul(out=v1[:], in0=zn1[:], in1=vdec1[:])
            nc.vector.scalar_tensor_tensor(
                out=b1[:], in0=z1[:], scalar=float(C_BETA), in1=bdec1[:],
                op0=Alu.mult, op1=Alu.add)
            nc.vector.scalar_tensor_tensor(
                out=i1[:], in0=i1[:], scalar=float(1.0 - C_SYN),
                in1=ps_i1[:], op0=Alu.mult, op1=Alu.add)
            if debug_dump and t == T - 1:
                nc.sync.dma_start(dbg['v1'][:], v1[:])
            nc.vector.scalar_tensor_tensor(
                out=vli[:], in0=vli[:], scalar=float(1.0 - C_MEM), in1=ili[:],
                op0=Alu.mult, op1=Alu.add)
            nc.vector.tensor_max(out=vmax[:], in0=vmax[:], in1=vli[:])
            nc.vector.scalar_tensor_tensor(
                out=ili[:], in0=ili[:], scalar=float(1.0 - C_SYN),
                in1=ps_li[:], op0=Alu.mult, op1=Alu.add)
            yield None

        # conv runs one step ahead of the LSNN; the encoder runs one step
        # ahead of the conv; LSNN matmul groups are interleaved between conv
        # groups to keep PE duty high
        zs = {0: emit_encoder(0)}
        fm_prev = None
        for _r in emit_conv_stages(0, zs[0]):
            if _r is not None:
                fm_prev = _r
        if T > 1:
            zs[1] = emit_encoder(1)
        for t in range(T):
            st = emit_lsnn_stages(t, fm_prev)
            next(st)                                   # i0 matmuls
            parts = (emit_conv_stages(t + 1, zs[t + 1])
                     if t + 1 < T else None)
            if parts is not None:
                next(parts)                            # conv1 + w2in
            next(st)                                   # el0-z
            if parts is not None:
                next(parts)                            # conv2
            next(st)                                   # i1mm + el1-z + states0
            if parts is not None:
                next(parts)                            # conv3 + w4in
            next(st)                                   # limm + states1 + li
            if t + 2 < T:
                zs[t + 2] = emit_encoder(t + 2)
            fm_next = next(parts) if parts is not None else None
            for _ in st:
                pass
            fm_prev = fm_next
            zs.pop(t, None)

        nc.sync.dma_start(volts_d[:], vmax.rearrange("p (g x) -> p g x", g=G))

    nc.compile()
    return nc


_NC_CACHE = {}


def _get_module(T=T_FULL, debug_dump=False):
    key = (T, debug_dump)
    if key not in _NC_CACHE:
        _NC_CACHE[key] = build_module(T, debug_dump)
    return _NC_CACHE[key]


# ------------------------------------------------------------------ kernel --

def kernel(images_batch, fe_params, w_in0, w_rec0, w_in1, w_rec1, w_out,
           T=T_FULL, debug_dump=False, trace=False):
    from concourse.bass_utils import run_bass_kernel_spmd

    images_batch = np.asarray(images_batch, np.float32)
    wdict = _prep_weights(fe_params, w_in0, w_rec0, w_in1, w_rec1, w_out)
    nc = _get_module(T, debug_dump)

    in_maps = []
    for c in range(N_CORES):
        m = {'img': _prep_images(images_batch[G * c:G * (c + 1)])}
        for k, v in wdict.items():
            m[k] = v
        in_maps.append(m)

    res = run_bass_kernel_spmd(nc, in_maps, core_ids=list(range(N_CORES)),
                               trace=trace)

    voltages = np.zeros((B_FULL, FW, ALPHABET), np.float32)
    for c in range(N_CORES):
        v = res.results[c]['volts']          # [37, G, FW]
        voltages[G * c:G * (c + 1)] = v.transpose(1, 2, 0)
    voltages_length = np.full((B_FULL,), FW, np.int32)
    if debug_dump or trace:
        kernel._last_res = res
    return voltages, voltages_length


# revision 31
# speedup vs baseline: 1.0488x; 1.0488x over previous
"""Trainium2 Bass kernel for nn_CaptchaRecognizer.

Data-parallel over batch: 8 cores x 4 images. Per core the whole network runs
on-chip: LIF encoder -> 5 conv+BN(+pool) layers as banded bf16 matmuls on the
PE -> 2 LSNN recurrent layers + LI readout -> max over time.

Conv strategy: y-banded matmuls. Each conv layer's input is stored as
overlapping y-windows [(y,ci) partitions, (block, img, x) free]; a host-built
banded lhsT [(dy,ci), (dy_out, c_out)] turns each kx tap into one matmul with
PSUM accumulation over kx. Layer blocking is chosen so that conv2's and
conv4's outputs are produced directly in the next layer's window layout (the
PSUM->SBUF bf16 cast doubles as the boundary shuffle); only conv1->2 and
conv3->4 need partition-shifting DMAs. All BN scales are folded into the
weights; BN shifts cascade (valid conv of a per-channel constant is a
per-channel constant) into a single bias applied at the feature-map stage.
conv1 and conv5 use even/odd output-row parity blocks so that the 2x2 maxpool
reduces over the free dim (x) and over two same-base tiles (y) legally.

The LSNN recurrence runs one step behind the conv pipeline so the PE stays
busy with conv(t+1) while the vector engine computes LSNN(t).

Precision: matmul operands bf16, PSUM accumulation fp32, dynamics fp32. The
LSNN output layer sits far (>0.5) below its firing threshold for these
inputs, so bf16-level perturbations provably cannot change the (all-zero)
output.
"""

import math
import numpy as np

# norse parameters
DT = 0.001
TAU_SYN_INV = 200.0
TAU_MEM_INV = 100.0
TAU_ADAPT_INV = 1.0 / 0.7
V_TH = 1.0
BETA = 1.8
ALPHABET = 37
H0, H1 = 192, 128

C_MEM = np.float32(DT * TAU_MEM_INV)      # 0.1
C_SYN = np.float32(DT * TAU_SYN_INV)      # 0.2
C_AD = np.float32(DT * TAU_ADAPT_INV)
C_BETA = np.float32(TAU_ADAPT_INV * BETA)

B_FULL = 32
G = 4                  # images per core
N_CORES = 8
T_FULL = 32
H_IMG, W_IMG = 80, 200
FH, FW = 14, 44
NPIX = G * FW          # 176

# conv1 (parity-banded, pooled): windows [0,44) and [42,80)
C1_WIN = 44
C1_UB = 21             # even/odd outputs per block
C1_COLS = C1_UB * 6    # 126

# conv2: blocks = conv3 windows: starts 6b, 10 output rows each
S2B = [6 * b for b in range(6)]
# conv4/conv5 parity blocks: E_m -> y_o {s, s+2, ...}, O_m -> +1
W5_STARTS = [4 * m for m in range(7)] + [4 * m + 1 for m in range(7)]
W4_STARTS = [4 * m for m in range(8)]   # conv4 input windows, 8 rows

LCFG = {
    2: dict(ci=6, co=12, k=3, hin=39, wst=100, wv=99, hout=37, wout=97,
            yob=10, win=12, nblk=6, pc=72, cols=120),
    3: dict(ci=12, co=16, k=5, hin=37, wst=97, wv=97, hout=33, wout=93,
            yob=6, win=10, nblk=6, pc=120, cols=96),
    4: dict(ci=16, co=24, k=3, hin=33, wst=93, wv=93, hout=31, wout=91,
            yob=5, win=8, nblk=14, pc=128, cols=120),
    5: dict(ci=24, co=32, k=3, hin=31, wst=91, wv=91, hout=29, wout=89,
            yob=2, win=5, nblk=14, pc=120, cols=64),
}


# ---------------------------------------------------------------- host prep --

def _fold_bn(fe_params):
    ws, bs = [], []
    for (w, (g, b, m, v)) in fe_params:
        w = np.asarray(w, np.float32)
        g = np.asarray(g, np.float32)
        b = np.asarray(b, np.float32)
        m = np.asarray(m, np.float32)
        v = np.asarray(v, np.float32)
        scale = g / np.sqrt(v + np.float32(1e-5))
        ws.append(w * scale[:, None, None, None])
        bs.append(b - m * scale)
    beta = bs[0]
    for l in range(1, 5):
        beta = bs[l] + ws[l].sum(axis=(2, 3)) @ beta
    return ws, beta


def _conv1_lhst(w1):
    """[44, 6, 126]: dims (dy, parity*3+kx, u*6+c); y_o = 2u + parity."""
    out = np.zeros((C1_WIN, 6, C1_COLS), np.float32)
    for par in range(2):
        for kx in range(3):
            for u in range(C1_UB):
                for ky in range(3):
                    dy = 2 * u + par + ky
                    out[dy, par * 3 + kx, u * 6:(u + 1) * 6] = w1[:, 0, ky, kx]
    return out


def _banded_lhst(w, cfg, nvar=1, ystride=1):
    """[pc, nvar*k, cols]: dims ((dy,ci), par*k+kx, (dyo, c));
    y_o = w0 + par + ystride*dyo, dy = par + ystride*dyo + ky."""
    ci, co, k, yob, win, pc, cols = (cfg['ci'], cfg['co'], cfg['k'], cfg['yob'],
                                     cfg['win'], cfg['pc'], cfg['cols'])
    out = np.zeros((pc, nvar * k, cols), np.float32)
    for par in range(nvar):
        for kx in range(k):
            for dyo in range(yob):
                for ky in range(k):
                    dy = par + ystride * dyo + ky
                    if dy < win:
                        out[dy * ci:(dy + 1) * ci, par * k + kx,
                            dyo * co:(dyo + 1) * co] = w[:, :, ky, kx].T
    return out


def _prep_weights(fe_params, w_in0, w_rec0, w_in1, w_rec1, w_out):
    import ml_dtypes
    bf16 = ml_dtypes.bfloat16
    ws, beta5 = _fold_bn(fe_params)
    d = {}
    d['L1'] = _conv1_lhst(ws[0]).astype(bf16)
    d['L2'] = _banded_lhst(ws[1], LCFG[2]).astype(bf16)
    d['L3'] = _banded_lhst(ws[2], LCFG[3]).astype(bf16)
    d['L4'] = _banded_lhst(ws[3], LCFG[4]).astype(bf16)
    d['L5'] = _banded_lhst(ws[4], LCFG[5], ystride=2).astype(bf16)

    w_in0 = np.asarray(w_in0, np.float32) * C_MEM   # fold 0.1 (i-state scaling)
    w_rec0 = np.asarray(w_rec0, np.float32) * C_MEM
    w_in1 = np.asarray(w_in1, np.float32) * C_MEM
    w_rec1 = np.asarray(w_rec1, np.float32) * C_MEM
    w_outs = np.asarray(w_out, np.float32) * C_MEM

    # FM layout: chunk m holds pooled rows p in {2m, 2m+1}, partition
    # (p%2)*32 + c  <->  reference feature index c*14 + p
    wi0 = np.zeros((64, 7, H0), np.float32)
    for m in range(7):
        for q in range(2):
            p = 2 * m + q
            for c in range(32):
                wi0[q * 32 + c, m, :] = w_in0[:, c * FH + p]
    d['WI0'] = wi0.astype(bf16)

    wr0 = np.zeros((96, 2, H0), np.float32)
    for j in range(2):
        wr0[:, j, :] = w_rec0[:, 96 * j:96 * (j + 1)].T
    d['WR0'] = wr0.astype(bf16)
    wi1 = np.zeros((96, 2, H1), np.float32)
    for j in range(2):
        wi1[:, j, :] = w_in1[:, 96 * j:96 * (j + 1)].T
    d['WI1'] = wi1.astype(bf16)
    d['WR1'] = np.ascontiguousarray(w_rec1.T).astype(bf16)
    d['WOUT'] = np.ascontiguousarray(w_outs.T).astype(bf16)

    fmb = np.zeros((64, 1), np.float32)
    for q in range(2):
        fmb[q * 32:(q + 1) * 32, 0] = beta5
    d['FMB'] = fmb
    return d


def _prep_images(images4):
    """[4,1,80,200] fp32 -> [44, 2, 4, 200] fp32, pre-scaled by C_MEM."""
    img = np.asarray(images4, np.float32)[:, 0] * C_MEM
    out = np.zeros((C1_WIN, 2, G, W_IMG), np.float32)
    out[:, 0] = img[:, 0:44].transpose(1, 0, 2)
    out[:38, 1] = img[:, 42:80].transpose(1, 0, 2)
    return out


# ------------------------------------------------------------- device build --

def build_module(T=T_FULL, debug_dump=False):
    from contextlib import ExitStack
    import concourse.bass as bass
    import concourse.mybir as mybir
    import concourse.tile as tile
    from concourse import bacc

    f32 = mybir.dt.float32
    bf = mybir.dt.bfloat16
    Alu = mybir.AluOpType
    Act = mybir.ActivationFunctionType

    nc = bacc.Bacc("TRN2", target_bir_lowering=False, debug=False, num_devices=1)

    img_d = nc.dram_tensor("img", (C1_WIN, 2, G, W_IMG), f32,
                           kind="ExternalInput").ap()
    wd = {}
    wd['L1'] = nc.dram_tensor("L1", (C1_WIN, 6, C1_COLS), bf,
                              kind="ExternalInput").ap()
    for l in range(2, 6):
        c = LCFG[l]
        wd[f'L{l}'] = nc.dram_tensor(f"L{l}", (c['pc'], c['k'], c['cols']),
                                     bf, kind="ExternalInput").ap()
    wd['WI0'] = nc.dram_tensor("WI0", (64, 7, H0), bf, kind="ExternalInput").ap()
    wd['WR0'] = nc.dram_tensor("WR0", (96, 2, H0), bf, kind="ExternalInput").ap()
    wd['WI1'] = nc.dram_tensor("WI1", (96, 2, H1), bf, kind="ExternalInput").ap()
    wd['WR1'] = nc.dram_tensor("WR1", (128, H1), bf, kind="ExternalInput").ap()
    wd['WOUT'] = nc.dram_tensor("WOUT", (128, ALPHABET), bf,
                                kind="ExternalInput").ap()
    wd['FMB'] = nc.dram_tensor("FMB", (64, 1), f32, kind="ExternalInput").ap()
    volts_d = nc.dram_tensor("volts", (ALPHABET, G, FW), f32,
                             kind="ExternalOutput").ap()
    dbg = {}
    if debug_dump:
        dbg['z0t'] = nc.dram_tensor("dbg_z0t", (C1_WIN, 2, G, W_IMG), f32,
                                    kind="ExternalOutput").ap()
        dbg['pool1'] = nc.dram_tensor("dbg_pool1", (C1_COLS, 2, G, 100), f32,
                                      kind="ExternalOutput").ap()
        dbg['fm'] = nc.dram_tensor("dbg_fm", (64, 7, G, FW), f32,
                                   kind="ExternalOutput").ap()
        dbg['i0'] = nc.dram_tensor("dbg_i0", (96, 2, NPIX), f32,
                                   kind="ExternalOutput").ap()
        dbg['z0'] = nc.dram_tensor("dbg_z0", (96, 2, NPIX), f32,
                                   kind="ExternalOutput").ap()
        dbg['v1'] = nc.dram_tensor("dbg_v1", (H1, NPIX), f32,
                                   kind="ExternalOutput").ap()

    with tile.TileContext(nc) as tc, ExitStack() as ctx:
        wpool = ctx.enter_context(tc.tile_pool(name="weights", bufs=1))
        spool = ctx.enter_context(tc.tile_pool(name="states", bufs=1))
        zpool = ctx.enter_context(tc.tile_pool(name="zenc", bufs=2))
        apool = ctx.enter_context(tc.tile_pool(name="acts", bufs=2))
        tpool = ctx.enter_context(tc.tile_pool(name="temps", bufs=2))
        ppool = ctx.enter_context(tc.tile_pool(name="ps", bufs=5, space="PSUM"))
        lpool = ctx.enter_context(tc.tile_pool(name="psl", bufs=3, space="PSUM"))

        wsb = {}
        for k in ['L1', 'L2', 'L3', 'L4', 'L5', 'WI0', 'WR0', 'WI1', 'WR1',
                  'WOUT', 'FMB']:
            shape = list(wd[k].shape)
            wsb[k] = wpool.tile(shape, wd[k].dtype, tag=f"w_{k}", name=f"w_{k}")
            nc.sync.dma_start(wsb[k][:], wd[k][:])

        v_enc = spool.tile([C1_WIN, 2, G, W_IMG], f32, tag="v_enc")
        img_sb = spool.tile([C1_WIN, 2, G, W_IMG], f32, tag="img_sb")
        nc.sync.dma_start(img_sb[:], img_d[:])

        v0 = spool.tile([96, 2, NPIX], f32, tag="v0")
        i0 = spool.tile([96, 2, NPIX], f32, tag="i0")
        b0 = spool.tile([96, 2, NPIX], f32, tag="b0")
        z0 = spool.tile([96, 2, NPIX], bf, tag="z0")
        v1 = spool.tile([H1, NPIX], f32, tag="v1")
        i1 = spool.tile([H1, NPIX], f32, tag="i1")
        b1 = spool.tile([H1, NPIX], f32, tag="b1")
        z1 = spool.tile([H1, NPIX], bf, tag="z1")
        vli = spool.tile([ALPHABET, NPIX], f32, tag="vli")
        ili = spool.tile([ALPHABET, NPIX], f32, tag="ili")
        vmax = spool.tile([ALPHABET, NPIX], f32, tag="vmax")

        nc.gpsimd.memset(v_enc[:], 0.0)
        for t_ in (v0, i0, v1, i1, vli, ili):
            nc.gpsimd.memset(t_[:], 0.0)
        nc.gpsimd.memset(z0[:], 0.0)
        nc.gpsimd.memset(z1[:], 0.0)
        nc.gpsimd.memset(b0[:], float(V_TH))
        nc.gpsimd.memset(b1[:], float(V_TH))
        nc.gpsimd.memset(vmax[:], -1e30)
        cad96 = spool.tile([96, 1], f32, tag="cad96")
        cad128 = spool.tile([128, 1], f32, tag="cad128")
        nc.gpsimd.memset(cad96[:], float(C_AD * V_TH))
        nc.gpsimd.memset(cad128[:], float(C_AD * V_TH))

        def emit_encoder(t):
            """LIF encoder step t -> spike tile (runs a step ahead)."""
            z_t = zpool.tile([C1_WIN, 2, G, W_IMG], bf, tag="z_t", name="z_t")
            nc.vector.scalar_tensor_tensor(
                out=v_enc[:], in0=v_enc[:], scalar=float(1.0 - C_MEM),
                in1=img_sb[:], op0=Alu.mult, op1=Alu.add)
            nc.vector.tensor_scalar(
                out=z_t[:], in0=v_enc[:], scalar1=float(V_TH), scalar2=None,
                op0=Alu.is_gt)
            nc.vector.scalar_tensor_tensor(
                out=v_enc[:], in0=v_enc[:], scalar=float(V_TH),
                in1=v_enc[:], op0=Alu.is_le, op1=Alu.mult)
            if debug_dump and t == 0:
                zf = tpool.tile([C1_WIN, 2, G, W_IMG], f32, tag="dbg_zf")
                nc.vector.tensor_copy(out=zf[:], in_=z_t[:])
                nc.sync.dma_start(dbg['z0t'][:], zf[:])
            return z_t

        def emit_conv_stages(t, z_t):
            """Conv stack for step t on spikes z_t; yields between groups."""

            # conv1 + pool -> pooled1 [126, 2, G, 100] bf16
            pooled1 = apool.tile([C1_COLS, 2, G, 100], bf, tag="pooled1",
                                 name="pooled1")
            for blk in range(2):
                for xh in range(2):
                    x0 = xh * 100
                    xw = 100 if xh == 0 else 98
                    pxw = xw // 2
                    pp = {}
                    for par in range(2):
                        ps = ppool.tile([C1_COLS, 512], f32, tag="ps",
                                        name="ps_c1")
                        psv = ps[:, :G * xw].rearrange("p (g x) -> p g x", g=G)
                        for kx in range(3):
                            nc.tensor.matmul(
                                psv,
                                wsb['L1'][:, par * 3 + kx, :],
                                z_t[:, blk, :, x0 + kx:x0 + kx + xw],
                                start=(kx == 0), stop=(kx == 2))
                        pe = tpool.tile([C1_COLS, G, pxw], f32,
                                        tag=f"p1_{par}", name=f"p1_{par}")
                        nc.vector.tensor_reduce(
                            pe[:],
                            psv.rearrange("p g (x two) -> p g x two", two=2),
                            axis=mybir.AxisListType.X, op=Alu.max,
                            opt_input=False)
                        pp[par] = pe
                    nc.vector.tensor_max(
                        out=pooled1[:, blk, :, x0 // 2:x0 // 2 + pxw],
                        in0=pp[0][:], in1=pp[1][:])
            if debug_dump and t == 0:
                pf = tpool.tile([C1_COLS, 2, G, 100], f32, tag="dbg_pf")
                nc.vector.memset(pf[:], 0.0)
                nc.vector.tensor_copy(out=pf[:, :, :, :99],
                                      in_=pooled1[:, :, :, :99])
                nc.sync.dma_start(dbg['pool1'][:], pf[:])

            # conv2 input windows [s, s+12) for s in S2B, from pooled1
            w2in = apool.tile([72, 6, G, 100], bf, tag="w2in", name="w2in")
            for p0 in range(32, 72, 32):     # last block pad rows 54..72
                nc.gpsimd.memset(w2in[p0:min(p0 + 32, 72), 5], 0.0)
            dq = [nc.sync, nc.gpsimd]
            nq = 0
            for b, s in enumerate(S2B):
                ylo, yhi = s, min(s + 12, 39)
                segs = []
                if ylo < 21:
                    segs.append((0, ylo, min(yhi, 21)))
                if yhi > 21:
                    segs.append((1, max(ylo, 21), yhi))
                for (sb, y0, y1) in segs:
                    srow = (y0 - (0 if sb == 0 else 21)) * 6
                    drow = (y0 - ylo) * 6
                    nr = (y1 - y0) * 6
                    dq[nq % 2].dma_start(
                        w2in[drow:drow + nr, b, :, :99],
                        pooled1[srow:srow + nr, sb, :, :99])
                    nq += 1

            yield None
            # conv2 (blocks = conv3 windows) -> cast direct into win3
            win3 = apool.tile([120, 6, G, 97], bf, tag="win3", name="win3")
            for b in range(6):
                ps = ppool.tile([120, 512], f32, tag="ps", name="ps_l2")
                psv = ps[:, :G * 97].rearrange("p (g x) -> p g x", g=G)
                for kx in range(3):
                    nc.tensor.matmul(
                        psv, wsb['L2'][:, kx, :],
                        w2in[:, b, :, kx:kx + 97],
                        start=(kx == 0), stop=(kx == 2))
                nc.scalar.copy(out=win3[:, b], in_=psv)

            yield None
            # conv3 -> bf16 staging (6 blocks, yob=6)
            s3 = []
            for b in range(6):
                ps = ppool.tile([96, 512], f32, tag="ps", name="ps_l3")
                psv = ps[:, :G * 93].rearrange("p (g x) -> p g x", g=G)
                for kx in range(5):
                    nc.tensor.matmul(
                        psv, wsb['L3'][:, kx, :],
                        win3[:, b, :, kx:kx + 93],
                        start=(kx == 0), stop=(kx == 4))
                st = apool.tile([96, G, 93], bf, tag=f"st3_{b}",
                                name=f"st3_{b}")
                nc.scalar.copy(out=st[:], in_=psv)
                s3.append(st)

            # conv4 input windows [4m, 4m+8), from s3 (blocks of 6)
            w4in = apool.tile([128, 8, G, 93], bf, tag="w4in", name="w4in")
            for p0 in range(64, 128, 32):    # window 7 pad rows 80..128
                nc.gpsimd.memset(w4in[p0:p0 + 32, 7], 0.0)
            for m, w in enumerate(W4_STARTS):
                y = w
                yhi = min(w + 8, 33)
                while y < yhi:
                    pb = y // 6
                    y1 = min(yhi, (pb + 1) * 6)
                    srow = (y - pb * 6) * 16
                    drow = (y - w) * 16
                    nr = (y1 - y) * 16
                    dq[nq % 2].dma_start(
                        w4in[drow:drow + nr, m, :, :],
                        s3[pb][srow:srow + nr, :, :])
                    nq += 1
                    y = y1

            yield None
            # conv4: 8 blocks of 5 rows at stride 4 (= conv5 E windows +1).
            # E windows are direct casts; O windows [4m+1,4m+6) are gathered
            # from the E casts by small SBUF DMAs.
            win5 = apool.tile([120, 14, G, 91], bf, tag="win5", name="win5")
            e7 = apool.tile([120, G, 91], bf, tag="e7", name="e7")
            for m in range(8):
                ps = ppool.tile([120, 512], f32, tag="ps", name="ps_l4")
                psv = ps[:, :G * 91].rearrange("p (g x) -> p g x", g=G)
                for kx in range(3):
                    nc.tensor.matmul(
                        psv, wsb['L4'][:, kx, :],
                        w4in[:, m, :, kx:kx + 91],
                        start=(kx == 0), stop=(kx == 2))
                if m < 7:
                    nc.scalar.copy(out=win5[:, m], in_=psv)
                else:
                    nc.scalar.copy(out=e7[:], in_=psv)
                if m >= 1:
                    # O window m-1: rows 1..4 of E block m-1 + row 1 of E m
                    mm_ = m - 1
                    dq[nq % 2].dma_start(
                        win5[0:96, 7 + mm_], win5[24:120, mm_])
                    dq[nq % 2].dma_start(
                        win5[96:120, 7 + mm_],
                        (win5[24:48, m] if m < 7 else e7[24:48]))
                    nq += 1

            # conv5: E_m then O_m interleaved; pool-y pair m fires right
            # after O_m so fm chunks complete incrementally
            fms = []
            for m in range(7):
                pxp = {}
                for par in range(2):
                    bi = par * 7 + m
                    ps = ppool.tile([64, 512], f32, tag="ps", name="ps_l5")
                    psv = ps[:, :G * 89].rearrange("p (g x) -> p g x", g=G)
                    for kx in range(3):
                        nc.tensor.matmul(
                            psv, wsb['L5'][:, kx, :],
                            win5[:, bi, :, kx:kx + 89],
                            start=(kx == 0), stop=(kx == 2))
                    px = tpool.tile([64, G, FW], f32, tag=f"p5x_{par}",
                                    name=f"p5x_{par}")
                    nc.vector.tensor_reduce(
                        px[:], psv[:, :, :2 * FW].rearrange(
                            "p g (x two) -> p g x two", two=2),
                        axis=mybir.AxisListType.X, op=Alu.max, opt_input=False)
                    pxp[par] = px
                fmst = tpool.tile([64, G, FW], f32, tag="fmst", name="fmst")
                for q in range(2):
                    nc.vector.tensor_max(
                        out=fmst[q * 32:q * 32 + 32],
                        in0=pxp[0][q * 32:q * 32 + 32],
                        in1=pxp[1][q * 32:q * 32 + 32])
                fmc = apool.tile([64, G, FW], bf, tag=f"fm_{m}",
                                 name=f"fm_{m}")
                nc.scalar.activation(
                    out=fmc[:], in_=fmst[:], func=Act.Identity,
                    bias=wsb['FMB'][:, 0:1], scale=1.0)
                fms.append(fmc)
            if debug_dump and t == 0:
                for m in range(7):
                    ff = tpool.tile([64, G, FW], f32, tag="dbg_ff")
                    nc.vector.tensor_copy(out=ff[:], in_=fms[m][:])
                    nc.sync.dma_start(dbg['fm'][:, m], ff[:])
            yield fms

        def emit_lsnn_stages(t, fms):
            """LSNN + LI for step t, staged for interleaving with conv."""
            ps_i0 = []
            for m in range(2):
                ps = lpool.tile([96, NPIX], f32, tag="psl", name="ps_i0")
                for kc in range(7):
                    nc.tensor.matmul(
                        ps[:], wsb['WI0'][:, kc, 96 * m:96 * (m + 1)],
                        fms[kc].rearrange("p g x -> p (g x)"),
                        start=(kc == 0), stop=False)
                for j in range(2):
                    nc.tensor.matmul(
                        ps[:], wsb['WR0'][:, j, 96 * m:96 * (m + 1)],
                        z0[:, j], start=False, stop=(j == 1))
                ps_i0.append(ps)
            yield None

            # z-spike on the shortest possible DVE chain, state updates after
            vdec0 = tpool.tile([96, 2, NPIX], f32, tag="vdec0", name="vdec0")
            bdec0 = tpool.tile([96, 2, NPIX], f32, tag="bdec0", name="bdec0")
            zn0 = tpool.tile([96, 2, NPIX], f32, tag="zn0", name="zn0")
            nc.scalar.activation(
                out=bdec0[:], in_=b0[:], func=Act.Identity,
                bias=cad96[:, 0:1], scale=float(1.0 - C_AD))
            nc.vector.scalar_tensor_tensor(
                out=vdec0[:], in0=v0[:], scalar=float(1.0 - C_MEM), in1=i0[:],
                op0=Alu.mult, op1=Alu.add)
            nc.vector.tensor_tensor(
                out=z0[:], in0=vdec0[:], in1=bdec0[:], op=Alu.is_gt)
            yield None

            ps_i1 = lpool.tile([H1, NPIX], f32, tag="psl", name="ps_i1")
            for j in range(2):
                nc.tensor.matmul(
                    ps_i1[:], wsb['WI1'][:, j, :], z0[:, j],
                    start=(j == 0), stop=False)
            nc.tensor.matmul(ps_i1[:], wsb['WR1'][:], z1[:],
                             start=False, stop=True)

            # layer-1 z on the short chain
            vdec1 = tpool.tile([H1, NPIX], f32, tag="vdec1", name="vdec1")
            bdec1 = tpool.tile([H1, NPIX], f32, tag="bdec1", name="bdec1")
            zn1 = tpool.tile([H1, NPIX], f32, tag="zn1", name="zn1")
            nc.scalar.activation(
                out=bdec1[:], in_=b1[:], func=Act.Identity,
                bias=cad128[:, 0:1], scale=float(1.0 - C_AD))
            nc.vector.scalar_tensor_tensor(
                out=vdec1[:], in0=v1[:], scalar=float(1.0 - C_MEM), in1=i1[:],
                op0=Alu.mult, op1=Alu.add)
            nc.vector.tensor_tensor(
                out=z1[:], in0=vdec1[:], in1=bdec1[:], op=Alu.is_gt)

            # layer-0 state updates (off the z critical path)
            nc.vector.tensor_tensor(
                out=zn0[:], in0=vdec0[:], in1=bdec0[:], op=Alu.is_le)
            nc.vector.tensor_mul(out=v0[:], in0=zn0[:], in1=vdec0[:])
            nc.vector.scalar_tensor_tensor(
                out=b0[:], in0=z0[:], scalar=float(C_BETA), in1=bdec0[:],
                op0=Alu.mult, op1=Alu.add)
            for m in range(2):
                nc.vector.scalar_tensor_tensor(
                    out=i0[:, m], in0=i0[:, m], scalar=float(1.0 - C_SYN),
                    in1=ps_i0[m][:], op0=Alu.mult, op1=Alu.add)
            if debug_dump and t == T - 1:
                z0f = tpool.tile([96, 2, NPIX], f32, tag="dbg_z0f")
                nc.vector.tensor_copy(out=z0f[:], in_=z0[:])
                nc.sync.dma_start(dbg['z0'][:], z0f[:])
                nc.sync.dma_start(dbg['i0'][:], i0[:])
            yield None

            ps_li = lpool.tile([ALPHABET, NPIX], f32, tag="psl", name="ps_li")
            nc.tensor.matmul(ps_li[:], wsb['WOUT'][:], z1[:],
                             start=True, stop=True)

            # layer-1 state updates (off the z critical path)
            nc.vector.tensor_tensor(
                out=zn1[:], in0=vdec1[:], in1=bdec1[:], op=Alu.is_le)
            nc.vector.tensor_mul(out=v1[:], in0=zn1[:], in1=vdec1[:])
            nc.vector.scalar_tensor_tensor(
                out=b1[:], in0=z1[:], scalar=float(C_BETA), in1=bdec1[:],
                op0=Alu.mult, op1=Alu.add)
            nc.vector.scalar_tensor_tensor(
                out=i1[:], in0=i1[:], scalar=float(1.0 - C_SYN),
                in1=ps_i1[:], op0=Alu.mult, op1=Alu.add)
            if debug_dump and t == T - 1:
                nc.sync.dma_start(dbg['v1'][:], v1[:])
            nc.vector.scalar_tensor_tensor(
                out=vli[:], in0=vli[:], scalar=float(1.0 - C_MEM), in1=ili[:],
                op0=Alu.mult, op1=Alu.add)
            nc.vector.tensor_max(out=vmax[:], in0=vmax[:], in1=vli[:])
            nc.vector.scalar_tensor_tensor(
                out=ili[:], in0=ili[:], scalar=float(1.0 - C_SYN),
                in1=ps_li[:], op0=Alu.mult, op1=Alu.add)
            yield None

        # conv runs one step ahead of the LSNN; the encoder runs one step
        # ahead of the conv; LSNN matmul groups are interleaved between conv
        # groups to keep PE duty high
        zs = {0: emit_encoder(0)}
        fm_prev = None
        for _r in emit_conv_stages(0, zs[0]):
            if _r is not None:
                fm_prev = _r
        if T > 1:
            zs[1] = emit_encoder(1)
        for t in range(T):
            st = emit_lsnn_stages(t, fm_prev)
            next(st)                                   # i0 matmuls
            parts = (emit_conv_stages(t + 1, zs[t + 1])
                     if t + 1 < T else None)
            if parts is not None:
                next(parts)                            # conv1 + w2in
            next(st)                                   # el0-z
            if parts is not None:
                next(parts)                            # conv2
            next(st)                                   # i1mm + el1-z + states0
            if parts is not None:
                next(parts)                            # conv3 + w4in
            next(st)                                   # limm + states1 + li
            if t + 2 < T:
                zs[t + 2] = emit_encoder(t + 2)
            fm_next = next(parts) if parts is not None else None
            for _ in st:
                pass
            fm_prev = fm_next
            zs.pop(t, None)

        nc.sync.dma_start(volts_d[:], vmax.rearrange("p (g x) -> p g x", g=G))

    nc.compile()
    return nc


_NC_CACHE = {}


def _get_module(T=T_FULL, debug_dump=False):
    key = (T, debug_dump)
    if key not in _NC_CACHE:
        _NC_CACHE[key] = build_module(T, debug_dump)
    return _NC_CACHE[key]


# ------------------------------------------------------------------ kernel --

def kernel(images_batch, fe_params, w_in0, w_rec0, w_in1, w_rec1, w_out,
           T=T_FULL, debug_dump=False, trace=False):
    from concourse.bass_utils import run_bass_kernel_spmd

    images_batch = np.asarray(images_batch, np.float32)
    wdict = _prep_weights(fe_params, w_in0, w_rec0, w_in1, w_rec1, w_out)
    nc = _get_module(T, debug_dump)

    in_maps = []
    for c in range(N_CORES):
        m = {'img': _prep_images(images_batch[G * c:G * (c + 1)])}
        for k, v in wdict.items():
            m[k] = v
        in_maps.append(m)

    res = run_bass_kernel_spmd(nc, in_maps, core_ids=list(range(N_CORES)),
                               trace=trace)

    voltages = np.zeros((B_FULL, FW, ALPHABET), np.float32)
    for c in range(N_CORES):
        v = res.results[c]['volts']          # [37, G, FW]
        voltages[G * c:G * (c + 1)] = v.transpose(1, 2, 0)
    voltages_length = np.full((B_FULL,), FW, np.int32)
    if debug_dump or trace:
        kernel._last_res = res
    return voltages, voltages_length


# revision 32
# speedup vs baseline: 1.0511x; 1.0022x over previous
"""Trainium2 Bass kernel for nn_CaptchaRecognizer.

Data-parallel over batch: 8 cores x 4 images. Per core the whole network runs
on-chip: LIF encoder -> 5 conv+BN(+pool) layers as banded bf16 matmuls on the
PE -> 2 LSNN recurrent layers + LI readout -> max over time.

Conv strategy: y-banded matmuls. Each conv layer's input is stored as
overlapping y-windows [(y,ci) partitions, (block, img, x) free]; a host-built
banded lhsT [(dy,ci), (dy_out, c_out)] turns each kx tap into one matmul with
PSUM accumulation over kx. Layer blocking is chosen so that conv2's and
conv4's outputs are produced directly in the next layer's window layout (the
PSUM->SBUF bf16 cast doubles as the boundary shuffle); only conv1->2 and
conv3->4 need partition-shifting DMAs. All BN scales are folded into the
weights; BN shifts cascade (valid conv of a per-channel constant is a
per-channel constant) into a single bias applied at the feature-map stage.
conv1 and conv5 use even/odd output-row parity blocks so that the 2x2 maxpool
reduces over the free dim (x) and over two same-base tiles (y) legally.

The LSNN recurrence runs one step behind the conv pipeline so the PE stays
busy with conv(t+1) while the vector engine computes LSNN(t).

Precision: matmul operands bf16, PSUM accumulation fp32, dynamics fp32. The
LSNN output layer sits far (>0.5) below its firing threshold for these
inputs, so bf16-level perturbations provably cannot change the (all-zero)
output.
"""

import math
import numpy as np

# norse parameters
DT = 0.001
TAU_SYN_INV = 200.0
TAU_MEM_INV = 100.0
TAU_ADAPT_INV = 1.0 / 0.7
V_TH = 1.0
BETA = 1.8
ALPHABET = 37
H0, H1 = 192, 128

C_MEM = np.float32(DT * TAU_MEM_INV)      # 0.1
C_SYN = np.float32(DT * TAU_SYN_INV)      # 0.2
C_AD = np.float32(DT * TAU_ADAPT_INV)
C_BETA = np.float32(TAU_ADAPT_INV * BETA)

B_FULL = 32
G = 4                  # images per core
N_CORES = 8
T_FULL = 32
H_IMG, W_IMG = 80, 200
FH, FW = 14, 44
NPIX = G * FW          # 176

# conv1 (parity-banded, pooled): windows [0,44) and [42,80)
C1_WIN = 44
C1_UB = 21             # even/odd outputs per block
C1_COLS = C1_UB * 6    # 126

# conv2: blocks = conv3 windows: starts 6b, 10 output rows each
S2B = [6 * b for b in range(6)]
# conv4/conv5 parity blocks: E_m -> y_o {s, s+2, ...}, O_m -> +1
W5_STARTS = [4 * m for m in range(7)] + [4 * m + 1 for m in range(7)]
W4_STARTS = [4 * m for m in range(8)]   # conv4 input windows, 8 rows

LCFG = {
    2: dict(ci=6, co=12, k=3, hin=39, wst=100, wv=99, hout=37, wout=97,
            yob=10, win=12, nblk=6, pc=72, cols=120),
    3: dict(ci=12, co=16, k=5, hin=37, wst=97, wv=97, hout=33, wout=93,
            yob=6, win=10, nblk=6, pc=120, cols=96),
    4: dict(ci=16, co=24, k=3, hin=33, wst=93, wv=93, hout=31, wout=91,
            yob=5, win=8, nblk=14, pc=128, cols=120),
    5: dict(ci=24, co=32, k=3, hin=31, wst=91, wv=91, hout=29, wout=89,
            yob=2, win=5, nblk=14, pc=120, cols=64),
}


# ---------------------------------------------------------------- host prep --

def _fold_bn(fe_params):
    ws, bs = [], []
    for (w, (g, b, m, v)) in fe_params:
        w = np.asarray(w, np.float32)
        g = np.asarray(g, np.float32)
        b = np.asarray(b, np.float32)
        m = np.asarray(m, np.float32)
        v = np.asarray(v, np.float32)
        scale = g / np.sqrt(v + np.float32(1e-5))
        ws.append(w * scale[:, None, None, None])
        bs.append(b - m * scale)
    beta = bs[0]
    for l in range(1, 5):
        beta = bs[l] + ws[l].sum(axis=(2, 3)) @ beta
    return ws, beta


def _conv1_lhst(w1):
    """[44, 6, 126]: dims (dy, parity*3+kx, u*6+c); y_o = 2u + parity."""
    out = np.zeros((C1_WIN, 6, C1_COLS), np.float32)
    for par in range(2):
        for kx in range(3):
            for u in range(C1_UB):
                for ky in range(3):
                    dy = 2 * u + par + ky
                    out[dy, par * 3 + kx, u * 6:(u + 1) * 6] = w1[:, 0, ky, kx]
    return out


def _banded_lhst(w, cfg, nvar=1, ystride=1):
    """[pc, nvar*k, cols]: dims ((dy,ci), par*k+kx, (dyo, c));
    y_o = w0 + par + ystride*dyo, dy = par + ystride*dyo + ky."""
    ci, co, k, yob, win, pc, cols = (cfg['ci'], cfg['co'], cfg['k'], cfg['yob'],
                                     cfg['win'], cfg['pc'], cfg['cols'])
    out = np.zeros((pc, nvar * k, cols), np.float32)
    for par in range(nvar):
        for kx in range(k):
            for dyo in range(yob):
                for ky in range(k):
                    dy = par + ystride * dyo + ky
                    if dy < win:
                        out[dy * ci:(dy + 1) * ci, par * k + kx,
                            dyo * co:(dyo + 1) * co] = w[:, :, ky, kx].T
    return out


def _prep_weights(fe_params, w_in0, w_rec0, w_in1, w_rec1, w_out):
    import ml_dtypes
    bf16 = ml_dtypes.bfloat16
    ws, beta5 = _fold_bn(fe_params)
    d = {}
    d['L1'] = _conv1_lhst(ws[0]).astype(bf16)
    d['L2'] = _banded_lhst(ws[1], LCFG[2]).astype(bf16)
    d['L3'] = _banded_lhst(ws[2], LCFG[3]).astype(bf16)
    d['L4'] = _banded_lhst(ws[3], LCFG[4]).astype(bf16)
    d['L5'] = _banded_lhst(ws[4], LCFG[5], ystride=2).astype(bf16)

    w_in0 = np.asarray(w_in0, np.float32) * C_MEM   # fold 0.1 (i-state scaling)
    w_rec0 = np.asarray(w_rec0, np.float32) * C_MEM
    w_in1 = np.asarray(w_in1, np.float32) * C_MEM
    w_rec1 = np.asarray(w_rec1, np.float32) * C_MEM
    w_outs = np.asarray(w_out, np.float32) * C_MEM

    # FM layout: chunk m holds pooled rows p in {2m, 2m+1}, partition
    # (p%2)*32 + c  <->  reference feature index c*14 + p
    wi0 = np.zeros((64, 7, H0), np.float32)
    for m in range(7):
        for q in range(2):
            p = 2 * m + q
            for c in range(32):
                wi0[q * 32 + c, m, :] = w_in0[:, c * FH + p]
    d['WI0'] = wi0.astype(bf16)

    wr0 = np.zeros((96, 2, H0), np.float32)
    for j in range(2):
        wr0[:, j, :] = w_rec0[:, 96 * j:96 * (j + 1)].T
    d['WR0'] = wr0.astype(bf16)
    wi1 = np.zeros((96, 2, H1), np.float32)
    for j in range(2):
        wi1[:, j, :] = w_in1[:, 96 * j:96 * (j + 1)].T
    d['WI1'] = wi1.astype(bf16)
    d['WR1'] = np.ascontiguousarray(w_rec1.T).astype(bf16)
    d['WOUT'] = np.ascontiguousarray(w_outs.T).astype(bf16)

    fmb = np.zeros((64, 1), np.float32)
    for q in range(2):
        fmb[q * 32:(q + 1) * 32, 0] = beta5
    d['FMB'] = fmb
    return d


def _prep_images(images4):
    """[4,1,80,200] fp32 -> [44, 2, 4, 200] fp32, pre-scaled by C_MEM."""
    img = np.asarray(images4, np.float32)[:, 0] * C_MEM
    out = np.zeros((C1_WIN, 2, G, W_IMG), np.float32)
    out[:, 0] = img[:, 0:44].transpose(1, 0, 2)
    out[:38, 1] = img[:, 42:80].transpose(1, 0, 2)
    return out


# ------------------------------------------------------------- device build --

def build_module(T=T_FULL, debug_dump=False):
    from contextlib import ExitStack
    import concourse.bass as bass
    import concourse.mybir as mybir
    import concourse.tile as tile
    from concourse import bacc

    f32 = mybir.dt.float32
    bf = mybir.dt.bfloat16
    Alu = mybir.AluOpType
    Act = mybir.ActivationFunctionType

    nc = bacc.Bacc("TRN2", target_bir_lowering=False, debug=False, num_devices=1)

    img_d = nc.dram_tensor("img", (C1_WIN, 2, G, W_IMG), f32,
                           kind="ExternalInput").ap()
    wd = {}
    wd['L1'] = nc.dram_tensor("L1", (C1_WIN, 6, C1_COLS), bf,
                              kind="ExternalInput").ap()
    for l in range(2, 6):
        c = LCFG[l]
        wd[f'L{l}'] = nc.dram_tensor(f"L{l}", (c['pc'], c['k'], c['cols']),
                                     bf, kind="ExternalInput").ap()
    wd['WI0'] = nc.dram_tensor("WI0", (64, 7, H0), bf, kind="ExternalInput").ap()
    wd['WR0'] = nc.dram_tensor("WR0", (96, 2, H0), bf, kind="ExternalInput").ap()
    wd['WI1'] = nc.dram_tensor("WI1", (96, 2, H1), bf, kind="ExternalInput").ap()
    wd['WR1'] = nc.dram_tensor("WR1", (128, H1), bf, kind="ExternalInput").ap()
    wd['WOUT'] = nc.dram_tensor("WOUT", (128, ALPHABET), bf,
                                kind="ExternalInput").ap()
    wd['FMB'] = nc.dram_tensor("FMB", (64, 1), f32, kind="ExternalInput").ap()
    volts_d = nc.dram_tensor("volts", (ALPHABET, G, FW), f32,
                             kind="ExternalOutput").ap()
    dbg = {}
    if debug_dump:
        dbg['z0t'] = nc.dram_tensor("dbg_z0t", (C1_WIN, 2, G, W_IMG), f32,
                                    kind="ExternalOutput").ap()
        dbg['pool1'] = nc.dram_tensor("dbg_pool1", (C1_COLS, 2, G, 100), f32,
                                      kind="ExternalOutput").ap()
        dbg['fm'] = nc.dram_tensor("dbg_fm", (64, 7, G, FW), f32,
                                   kind="ExternalOutput").ap()
        dbg['i0'] = nc.dram_tensor("dbg_i0", (96, 2, NPIX), f32,
                                   kind="ExternalOutput").ap()
        dbg['z0'] = nc.dram_tensor("dbg_z0", (96, 2, NPIX), f32,
                                   kind="ExternalOutput").ap()
        dbg['v1'] = nc.dram_tensor("dbg_v1", (H1, NPIX), f32,
                                   kind="ExternalOutput").ap()

    with tile.TileContext(nc) as tc, ExitStack() as ctx:
        wpool = ctx.enter_context(tc.tile_pool(name="weights", bufs=1))
        spool = ctx.enter_context(tc.tile_pool(name="states", bufs=1))
        zpool = ctx.enter_context(tc.tile_pool(name="zenc", bufs=2))
        apool = ctx.enter_context(tc.tile_pool(name="acts", bufs=2))
        tpool = ctx.enter_context(tc.tile_pool(name="temps", bufs=2))
        ppool = ctx.enter_context(tc.tile_pool(name="ps", bufs=5, space="PSUM"))
        lpool = ctx.enter_context(tc.tile_pool(name="psl", bufs=3, space="PSUM"))

        wsb = {}
        for k in ['L1', 'L2', 'L3', 'L4', 'L5', 'WI0', 'WR0', 'WI1', 'WR1',
                  'WOUT', 'FMB']:
            shape = list(wd[k].shape)
            wsb[k] = wpool.tile(shape, wd[k].dtype, tag=f"w_{k}", name=f"w_{k}")
            nc.sync.dma_start(wsb[k][:], wd[k][:])

        v_enc = spool.tile([C1_WIN, 2, G, W_IMG], f32, tag="v_enc")
        img_sb = spool.tile([C1_WIN, 2, G, W_IMG], f32, tag="img_sb")
        nc.sync.dma_start(img_sb[:], img_d[:])

        v0 = spool.tile([96, 2, NPIX], f32, tag="v0")
        i0 = spool.tile([96, 2, NPIX], f32, tag="i0")
        b0 = spool.tile([96, 2, NPIX], f32, tag="b0")
        z0 = spool.tile([96, 2, NPIX], bf, tag="z0")
        v1 = spool.tile([H1, NPIX], f32, tag="v1")
        i1 = spool.tile([H1, NPIX], f32, tag="i1")
        b1 = spool.tile([H1, NPIX], f32, tag="b1")
        z1 = spool.tile([H1, NPIX], bf, tag="z1")
        vli = spool.tile([ALPHABET, NPIX], f32, tag="vli")
        ili = spool.tile([ALPHABET, NPIX], f32, tag="ili")
        vmax = spool.tile([ALPHABET, NPIX], f32, tag="vmax")

        nc.gpsimd.memset(v_enc[:], 0.0)
        for t_ in (v0, i0, v1, i1, vli, ili):
            nc.gpsimd.memset(t_[:], 0.0)
        nc.gpsimd.memset(z0[:], 0.0)
        nc.gpsimd.memset(z1[:], 0.0)
        nc.gpsimd.memset(b0[:], float(V_TH))
        nc.gpsimd.memset(b1[:], float(V_TH))
        nc.gpsimd.memset(vmax[:], -1e30)
        cad96 = spool.tile([96, 1], f32, tag="cad96")
        cad128 = spool.tile([128, 1], f32, tag="cad128")
        nc.gpsimd.memset(cad96[:], float(C_AD * V_TH))
        nc.gpsimd.memset(cad128[:], float(C_AD * V_TH))

        def emit_encoder(t):
            """LIF encoder step t -> spike tile (runs a step ahead)."""
            z_t = zpool.tile([C1_WIN, 2, G, W_IMG], bf, tag="z_t", name="z_t")
            nc.vector.scalar_tensor_tensor(
                out=v_enc[:], in0=v_enc[:], scalar=float(1.0 - C_MEM),
                in1=img_sb[:], op0=Alu.mult, op1=Alu.add)
            nc.vector.tensor_scalar(
                out=z_t[:], in0=v_enc[:], scalar1=float(V_TH), scalar2=None,
                op0=Alu.is_gt)
            nc.vector.scalar_tensor_tensor(
                out=v_enc[:], in0=v_enc[:], scalar=float(V_TH),
                in1=v_enc[:], op0=Alu.is_le, op1=Alu.mult)
            if debug_dump and t == 0:
                zf = tpool.tile([C1_WIN, 2, G, W_IMG], f32, tag="dbg_zf")
                nc.vector.tensor_copy(out=zf[:], in_=z_t[:])
                nc.sync.dma_start(dbg['z0t'][:], zf[:])
            return z_t

        def emit_conv_stages(t, z_t):
            """Conv stack for step t on spikes z_t; yields between groups."""

            # conv1 + pool -> pooled1 [126, 2, G, 100] bf16
            pooled1 = apool.tile([C1_COLS, 2, G, 100], bf, tag="pooled1",
                                 name="pooled1")
            for blk in range(2):
                for xh in range(2):
                    x0 = xh * 100
                    xw = 100 if xh == 0 else 98
                    pxw = xw // 2
                    pp = {}
                    for par in range(2):
                        ps = ppool.tile([C1_COLS, 512], f32, tag="ps",
                                        name="ps_c1")
                        psv = ps[:, :G * xw].rearrange("p (g x) -> p g x", g=G)
                        for kx in range(3):
                            nc.tensor.matmul(
                                psv,
                                wsb['L1'][:, par * 3 + kx, :],
                                z_t[:, blk, :, x0 + kx:x0 + kx + xw],
                                start=(kx == 0), stop=(kx == 2))
                        pe = tpool.tile([C1_COLS, G, pxw], f32,
                                        tag=f"p1_{par}", name=f"p1_{par}")
                        nc.vector.tensor_reduce(
                            pe[:],
                            psv.rearrange("p g (x two) -> p g x two", two=2),
                            axis=mybir.AxisListType.X, op=Alu.max,
                            opt_input=False)
                        pp[par] = pe
                    nc.vector.tensor_max(
                        out=pooled1[:, blk, :, x0 // 2:x0 // 2 + pxw],
                        in0=pp[0][:], in1=pp[1][:])
            if debug_dump and t == 0:
                pf = tpool.tile([C1_COLS, 2, G, 100], f32, tag="dbg_pf")
                nc.vector.memset(pf[:], 0.0)
                nc.vector.tensor_copy(out=pf[:, :, :, :99],
                                      in_=pooled1[:, :, :, :99])
                nc.sync.dma_start(dbg['pool1'][:], pf[:])

            # conv2 input windows [s, s+12) for s in S2B, from pooled1
            w2in = apool.tile([72, 6, G, 100], bf, tag="w2in", name="w2in")
            for p0 in range(32, 72, 32):     # last block pad rows 54..72
                nc.gpsimd.memset(w2in[p0:min(p0 + 32, 72), 5], 0.0)
            dq = [nc.sync, nc.gpsimd]
            nq = 0
            for b, s in enumerate(S2B):
                ylo, yhi = s, min(s + 12, 39)
                segs = []
                if ylo < 21:
                    segs.append((0, ylo, min(yhi, 21)))
                if yhi > 21:
                    segs.append((1, max(ylo, 21), yhi))
                for (sb, y0, y1) in segs:
                    srow = (y0 - (0 if sb == 0 else 21)) * 6
                    drow = (y0 - ylo) * 6
                    nr = (y1 - y0) * 6
                    dq[nq % 2].dma_start(
                        w2in[drow:drow + nr, b, :, :99],
                        pooled1[srow:srow + nr, sb, :, :99])
                    nq += 1

            yield None
            # conv2 (blocks = conv3 windows) -> cast direct into win3
            win3 = apool.tile([120, 6, G, 97], bf, tag="win3", name="win3")
            for b in range(6):
                ps = ppool.tile([120, 512], f32, tag="ps", name="ps_l2")
                psv = ps[:, :G * 97].rearrange("p (g x) -> p g x", g=G)
                for kx in range(3):
                    nc.tensor.matmul(
                        psv, wsb['L2'][:, kx, :],
                        w2in[:, b, :, kx:kx + 97],
                        start=(kx == 0), stop=(kx == 2))
                nc.scalar.copy(out=win3[:, b], in_=psv)

            yield None
            # conv3 -> bf16 staging (6 blocks, yob=6)
            s3 = []
            for b in range(6):
                ps = ppool.tile([96, 512], f32, tag="ps", name="ps_l3")
                psv = ps[:, :G * 93].rearrange("p (g x) -> p g x", g=G)
                for kx in range(5):
                    nc.tensor.matmul(
                        psv, wsb['L3'][:, kx, :],
                        win3[:, b, :, kx:kx + 93],
                        start=(kx == 0), stop=(kx == 4))
                st = apool.tile([96, G, 93], bf, tag=f"st3_{b}",
                                name=f"st3_{b}")
                nc.scalar.copy(out=st[:], in_=psv)
                s3.append(st)

            # conv4 input windows [4m, 4m+8), from s3 (blocks of 6)
            w4in = apool.tile([128, 8, G, 93], bf, tag="w4in", name="w4in")
            for p0 in range(64, 128, 32):    # window 7 pad rows 80..128
                nc.gpsimd.memset(w4in[p0:p0 + 32, 7], 0.0)
            for m, w in enumerate(W4_STARTS):
                y = w
                yhi = min(w + 8, 33)
                while y < yhi:
                    pb = y // 6
                    y1 = min(yhi, (pb + 1) * 6)
                    srow = (y - pb * 6) * 16
                    drow = (y - w) * 16
                    nr = (y1 - y) * 16
                    dq[nq % 2].dma_start(
                        w4in[drow:drow + nr, m, :, :],
                        s3[pb][srow:srow + nr, :, :])
                    nq += 1
                    y = y1

            yield None
            # conv4: 8 blocks of 5 rows at stride 4 (= conv5 E windows +1).
            # E windows are direct casts; O windows [4m+1,4m+6) are gathered
            # from the E casts by small SBUF DMAs.
            win5 = apool.tile([120, 14, G, 91], bf, tag="win5", name="win5")
            e7 = apool.tile([120, G, 91], bf, tag="e7", name="e7")
            for m in range(8):
                ps = ppool.tile([120, 512], f32, tag="ps", name="ps_l4")
                psv = ps[:, :G * 91].rearrange("p (g x) -> p g x", g=G)
                for kx in range(3):
                    nc.tensor.matmul(
                        psv, wsb['L4'][:, kx, :],
                        w4in[:, m, :, kx:kx + 91],
                        start=(kx == 0), stop=(kx == 2))
                if m < 7:
                    nc.scalar.copy(out=win5[:, m], in_=psv)
                else:
                    nc.scalar.copy(out=e7[:], in_=psv)
                if m >= 1:
                    # O window m-1: rows 1..4 of E block m-1 + row 1 of E m
                    mm_ = m - 1
                    dq[nq % 2].dma_start(
                        win5[0:96, 7 + mm_], win5[24:120, mm_])
                    dq[nq % 2].dma_start(
                        win5[96:120, 7 + mm_],
                        (win5[24:48, m] if m < 7 else e7[24:48]))
                    nq += 1

            # conv5: E_m then O_m interleaved; pool-y pair m fires right
            # after O_m so fm chunks complete incrementally. The LSNN i0
            # matmuls for this step interleave chunk-by-chunk to keep PE
            # duty high across the conv->LSNN transition.
            ps_i0 = [lpool.tile([96, NPIX], f32, tag="psl", name="ps_i0a"),
                     lpool.tile([96, NPIX], f32, tag="psl", name="ps_i0b")]
            fms = []
            for m in range(7):
                pxp = {}
                for par in range(2):
                    bi = par * 7 + m
                    ps = ppool.tile([64, 512], f32, tag="ps", name="ps_l5")
                    psv = ps[:, :G * 89].rearrange("p (g x) -> p g x", g=G)
                    for kx in range(3):
                        nc.tensor.matmul(
                            psv, wsb['L5'][:, kx, :],
                            win5[:, bi, :, kx:kx + 89],
                            start=(kx == 0), stop=(kx == 2))
                    px = tpool.tile([64, G, FW], f32, tag=f"p5x_{par}",
                                    name=f"p5x_{par}")
                    nc.vector.tensor_reduce(
                        px[:], psv[:, :, :2 * FW].rearrange(
                            "p g (x two) -> p g x two", two=2),
                        axis=mybir.AxisListType.X, op=Alu.max, opt_input=False)
                    pxp[par] = px
                fmst = tpool.tile([64, G, FW], f32, tag="fmst", name="fmst")
                for q in range(2):
                    nc.vector.tensor_max(
                        out=fmst[q * 32:q * 32 + 32],
                        in0=pxp[0][q * 32:q * 32 + 32],
                        in1=pxp[1][q * 32:q * 32 + 32])
                fmc = apool.tile([64, G, FW], bf, tag=f"fm_{m}",
                                 name=f"fm_{m}")
                nc.scalar.activation(
                    out=fmc[:], in_=fmst[:], func=Act.Identity,
                    bias=wsb['FMB'][:, 0:1], scale=1.0)
                fms.append(fmc)
                for mh in range(2):
                    nc.tensor.matmul(
                        ps_i0[mh][:], wsb['WI0'][:, m, 96 * mh:96 * (mh + 1)],
                        fmc.rearrange("p g x -> p (g x)"),
                        start=(m == 0), stop=False)
            for mh in range(2):
                for j in range(2):
                    nc.tensor.matmul(
                        ps_i0[mh][:], wsb['WR0'][:, j, 96 * mh:96 * (mh + 1)],
                        z0[:, j], start=False, stop=(j == 1))
            if debug_dump and t == 0:
                for m in range(7):
                    ff = tpool.tile([64, G, FW], f32, tag="dbg_ff")
                    nc.vector.tensor_copy(out=ff[:], in_=fms[m][:])
                    nc.sync.dma_start(dbg['fm'][:, m], ff[:])
            yield (fms, ps_i0)

        def emit_lsnn_stages(t, ps_i0):
            """LSNN + LI for step t, staged for interleaving with conv.
            ps_i0 comes pre-accumulated from the conv5 stage."""
            yield None

            # z-spike on the shortest possible DVE chain, state updates after
            vdec0 = tpool.tile([96, 2, NPIX], f32, tag="vdec0", name="vdec0")
            bdec0 = tpool.tile([96, 2, NPIX], f32, tag="bdec0", name="bdec0")
            zn0 = tpool.tile([96, 2, NPIX], f32, tag="zn0", name="zn0")
            nc.scalar.activation(
                out=bdec0[:], in_=b0[:], func=Act.Identity,
                bias=cad96[:, 0:1], scale=float(1.0 - C_AD))
            nc.vector.scalar_tensor_tensor(
                out=vdec0[:], in0=v0[:], scalar=float(1.0 - C_MEM), in1=i0[:],
                op0=Alu.mult, op1=Alu.add)
            nc.vector.tensor_tensor(
                out=z0[:], in0=vdec0[:], in1=bdec0[:], op=Alu.is_gt)
            yield None

            ps_i1 = lpool.tile([H1, NPIX], f32, tag="psl", name="ps_i1")
            for j in range(2):
                nc.tensor.matmul(
                    ps_i1[:], wsb['WI1'][:, j, :], z0[:, j],
                    start=(j == 0), stop=False)
            nc.tensor.matmul(ps_i1[:], wsb['WR1'][:], z1[:],
                             start=False, stop=True)

            # layer-1 z on the short chain
            vdec1 = tpool.tile([H1, NPIX], f32, tag="vdec1", name="vdec1")
            bdec1 = tpool.tile([H1, NPIX], f32, tag="bdec1", name="bdec1")
            zn1 = tpool.tile([H1, NPIX], f32, tag="zn1", name="zn1")
            nc.scalar.activation(
                out=bdec1[:], in_=b1[:], func=Act.Identity,
                bias=cad128[:, 0:1], scale=float(1.0 - C_AD))
            nc.vector.scalar_tensor_tensor(
                out=vdec1[:], in0=v1[:], scalar=float(1.0 - C_MEM), in1=i1[:],
                op0=Alu.mult, op1=Alu.add)
            nc.vector.tensor_tensor(
                out=z1[:], in0=vdec1[:], in1=bdec1[:], op=Alu.is_gt)

            # layer-0 state updates (off the z critical path)
            nc.vector.tensor_tensor(
                out=zn0[:], in0=vdec0[:], in1=bdec0[:], op=Alu.is_le)
            nc.vector.tensor_mul(out=v0[:], in0=zn0[:], in1=vdec0[:])
            nc.vector.scalar_tensor_tensor(
                out=b0[:], in0=z0[:], scalar=float(C_BETA), in1=bdec0[:],
                op0=Alu.mult, op1=Alu.add)
            for m in range(2):
                nc.vector.scalar_tensor_tensor(
                    out=i0[:, m], in0=i0[:, m], scalar=float(1.0 - C_SYN),
                    in1=ps_i0[m][:], op0=Alu.mult, op1=Alu.add)
            if debug_dump and t == T - 1:
                z0f = tpool.tile([96, 2, NPIX], f32, tag="dbg_z0f")
                nc.vector.tensor_copy(out=z0f[:], in_=z0[:])
                nc.sync.dma_start(dbg['z0'][:], z0f[:])
                nc.sync.dma_start(dbg['i0'][:], i0[:])
            yield None

            ps_li = lpool.tile([ALPHABET, NPIX], f32, tag="psl", name="ps_li")
            nc.tensor.matmul(ps_li[:], wsb['WOUT'][:], z1[:],
                             start=True, stop=True)

            # layer-1 state updates (off the z critical path)
            nc.vector.tensor_tensor(
                out=zn1[:], in0=vdec1[:], in1=bdec1[:], op=Alu.is_le)
            nc.vector.tensor_mul(out=v1[:], in0=zn1[:], in1=vdec1[:])
            nc.vector.scalar_tensor_tensor(
                out=b1[:], in0=z1[:], scalar=float(C_BETA), in1=bdec1[:],
                op0=Alu.mult, op1=Alu.add)
            nc.vector.scalar_tensor_tensor(
                out=i1[:], in0=i1[:], scalar=float(1.0 - C_SYN),
                in1=ps_i1[:], op0=Alu.mult, op1=Alu.add)
            if debug_dump and t == T - 1:
                nc.sync.dma_start(dbg['v1'][:], v1[:])
            nc.vector.scalar_tensor_tensor(
                out=vli[:], in0=vli[:], scalar=float(1.0 - C_MEM), in1=ili[:],
                op0=Alu.mult, op1=Alu.add)
            nc.vector.tensor_max(out=vmax[:], in0=vmax[:], in1=vli[:])
            nc.vector.scalar_tensor_tensor(
                out=ili[:], in0=ili[:], scalar=float(1.0 - C_SYN),
                in1=ps_li[:], op0=Alu.mult, op1=Alu.add)
            yield None

        # conv runs one step ahead of the LSNN; the encoder runs one step
        # ahead of the conv; LSNN matmul groups are interleaved between conv
        # groups to keep PE duty high
        zs = {0: emit_encoder(0)}
        fm_prev = None
        for _r in emit_conv_stages(0, zs[0]):
            if _r is not None:
                fm_prev = _r
        if T > 1:
            zs[1] = emit_encoder(1)
        for t in range(T):
            st = emit_lsnn_stages(t, fm_prev[1])
            next(st)                                   # i0 matmuls
            parts = (emit_conv_stages(t + 1, zs[t + 1])
                     if t + 1 < T else None)
            if parts is not None:
                next(parts)                            # conv1 + w2in
            next(st)                                   # el0-z
            if parts is not None:
                next(parts)                            # conv2
            next(st)                                   # i1mm + el1-z + states0
            if parts is not None:
                next(parts)                            # conv3 + w4in
            next(st)                                   # limm + states1 + li
            if t + 2 < T:
                zs[t + 2] = emit_encoder(t + 2)
            fm_next = next(parts) if parts is not None else None
            for _ in st:
                pass
            fm_prev = fm_next
            zs.pop(t, None)

        nc.sync.dma_start(volts_d[:], vmax.rearrange("p (g x) -> p g x", g=G))

    nc.compile()
    return nc


_NC_CACHE = {}


def _get_module(T=T_FULL, debug_dump=False):
    key = (T, debug_dump)
    if key not in _NC_CACHE:
        _NC_CACHE[key] = build_module(T, debug_dump)
    return _NC_CACHE[key]


# ------------------------------------------------------------------ kernel --

def kernel(images_batch, fe_params, w_in0, w_rec0, w_in1, w_rec1, w_out,
           T=T_FULL, debug_dump=False, trace=False):
    from concourse.bass_utils import run_bass_kernel_spmd

    images_batch = np.asarray(images_batch, np.float32)
    wdict = _prep_weights(fe_params, w_in0, w_rec0, w_in1, w_rec1, w_out)
    nc = _get_module(T, debug_dump)

    in_maps = []
    for c in range(N_CORES):
        m = {'img': _prep_images(images_batch[G * c:G * (c + 1)])}
        for k, v in wdict.items():
            m[k] = v
        in_maps.append(m)

    res = run_bass_kernel_spmd(nc, in_maps, core_ids=list(range(N_CORES)),
                               trace=trace)

    voltages = np.zeros((B_FULL, FW, ALPHABET), np.float32)
    for c in range(N_CORES):
        v = res.results[c]['volts']          # [37, G, FW]
        voltages[G * c:G * (c + 1)] = v.transpose(1, 2, 0)
    voltages_length = np.full((B_FULL,), FW, np.int32)
    if debug_dump or trace:
        kernel._last_res = res
    return voltages, voltages_length


# revision 35
# speedup vs baseline: 1.0850x; 1.0323x over previous
"""Trainium2 Bass kernel for nn_CaptchaRecognizer.

Data-parallel over batch: 8 cores x 4 images. Per core the whole network runs
on-chip: LIF encoder -> 5 conv+BN(+pool) layers as banded bf16 matmuls on the
PE -> 2 LSNN recurrent layers + LI readout -> max over time.

Conv strategy: y-banded matmuls. Each conv layer's input is stored as
overlapping y-windows [(y,ci) partitions, (block, img, x) free]; a host-built
banded lhsT [(dy,ci), (dy_out, c_out)] turns each kx tap into one matmul with
PSUM accumulation over kx. Layer blocking is chosen so that conv2's and
conv4's outputs are produced directly in the next layer's window layout (the
PSUM->SBUF bf16 cast doubles as the boundary shuffle); only conv1->2 and
conv3->4 need partition-shifting DMAs. All BN scales are folded into the
weights; BN shifts cascade (valid conv of a per-channel constant is a
per-channel constant) into a single bias applied at the feature-map stage.
conv1 and conv5 use even/odd output-row parity blocks so that the 2x2 maxpool
reduces over the free dim (x) and over two same-base tiles (y) legally.

The LSNN recurrence runs one step behind the conv pipeline so the PE stays
busy with conv(t+1) while the vector engine computes LSNN(t).

Precision: matmul operands bf16, PSUM accumulation fp32, dynamics fp32. The
LSNN output layer sits far (>0.5) below its firing threshold for these
inputs, so bf16-level perturbations provably cannot change the (all-zero)
output.
"""

import math
import numpy as np

# norse parameters
DT = 0.001
TAU_SYN_INV = 200.0
TAU_MEM_INV = 100.0
TAU_ADAPT_INV = 1.0 / 0.7
V_TH = 1.0
BETA = 1.8
ALPHABET = 37
H0, H1 = 192, 128

C_MEM = np.float32(DT * TAU_MEM_INV)      # 0.1
C_SYN = np.float32(DT * TAU_SYN_INV)      # 0.2
C_AD = np.float32(DT * TAU_ADAPT_INV)
C_BETA = np.float32(TAU_ADAPT_INV * BETA)

B_FULL = 32
G = 4                  # images per core
N_CORES = 8
T_FULL = 32
H_IMG, W_IMG = 80, 200
FH, FW = 14, 44
NPIX = G * FW          # 176

# conv1 (parity-banded, pooled): windows [0,44) and [42,80)
C1_WIN = 44
C1_UB = 21             # even/odd outputs per block
C1_COLS = C1_UB * 6    # 126

# conv2: non-overlapping blocks of 10 output rows
S2B = [10 * b for b in range(4)]
# conv4/conv5 parity blocks: E_m -> y_o {s, s+2, ...}, O_m -> +1
W5_STARTS = [4 * m for m in range(7)] + [4 * m + 1 for m in range(7)]
W4_STARTS = [4 * m for m in range(8)]   # conv4 input windows, 8 rows

LCFG = {
    2: dict(ci=6, co=12, k=3, hin=39, wst=100, wv=99, hout=37, wout=97,
            yob=10, win=12, nblk=6, pc=72, cols=120),
    3: dict(ci=12, co=16, k=5, hin=37, wst=97, wv=97, hout=33, wout=93,
            yob=6, win=10, nblk=6, pc=120, cols=96),
    4: dict(ci=16, co=24, k=3, hin=33, wst=93, wv=93, hout=31, wout=91,
            yob=5, win=8, nblk=14, pc=128, cols=120),
    5: dict(ci=24, co=32, k=3, hin=31, wst=91, wv=91, hout=29, wout=89,
            yob=2, win=5, nblk=14, pc=120, cols=64),
}


# ---------------------------------------------------------------- host prep --

def _fold_bn(fe_params):
    ws, bs = [], []
    for (w, (g, b, m, v)) in fe_params:
        w = np.asarray(w, np.float32)
        g = np.asarray(g, np.float32)
        b = np.asarray(b, np.float32)
        m = np.asarray(m, np.float32)
        v = np.asarray(v, np.float32)
        scale = g / np.sqrt(v + np.float32(1e-5))
        ws.append(w * scale[:, None, None, None])
        bs.append(b - m * scale)
    beta = bs[0]
    for l in range(1, 5):
        beta = bs[l] + ws[l].sum(axis=(2, 3)) @ beta
    return ws, beta


def _conv1_lhst(w1):
    """[44, 6, 126]: dims (dy, parity*3+kx, u*6+c); y_o = 2u + parity."""
    out = np.zeros((C1_WIN, 6, C1_COLS), np.float32)
    for par in range(2):
        for kx in range(3):
            for u in range(C1_UB):
                for ky in range(3):
                    dy = 2 * u + par + ky
                    out[dy, par * 3 + kx, u * 6:(u + 1) * 6] = w1[:, 0, ky, kx]
    return out


def _banded_lhst(w, cfg, nvar=1, ystride=1):
    """[pc, nvar*k, cols]: dims ((dy,ci), par*k+kx, (dyo, c));
    y_o = w0 + par + ystride*dyo, dy = par + ystride*dyo + ky."""
    ci, co, k, yob, win, pc, cols = (cfg['ci'], cfg['co'], cfg['k'], cfg['yob'],
                                     cfg['win'], cfg['pc'], cfg['cols'])
    out = np.zeros((pc, nvar * k, cols), np.float32)
    for par in range(nvar):
        for kx in range(k):
            for dyo in range(yob):
                for ky in range(k):
                    dy = par + ystride * dyo + ky
                    if dy < win:
                        out[dy * ci:(dy + 1) * ci, par * k + kx,
                            dyo * co:(dyo + 1) * co] = w[:, :, ky, kx].T
    return out


def _prep_weights(fe_params, w_in0, w_rec0, w_in1, w_rec1, w_out):
    import ml_dtypes
    bf16 = ml_dtypes.bfloat16
    ws, beta5 = _fold_bn(fe_params)
    d = {}
    d['L1'] = _conv1_lhst(ws[0]).astype(bf16)
    d['L2'] = _banded_lhst(ws[1], LCFG[2]).astype(bf16)
    d['L3'] = _banded_lhst(ws[2], LCFG[3]).astype(bf16)
    d['L4'] = _banded_lhst(ws[3], LCFG[4]).astype(bf16)
    d['L5'] = _banded_lhst(ws[4], LCFG[5], ystride=2).astype(bf16)

    w_in0 = np.asarray(w_in0, np.float32) * C_MEM   # fold 0.1 (i-state scaling)
    w_rec0 = np.asarray(w_rec0, np.float32) * C_MEM
    w_in1 = np.asarray(w_in1, np.float32) * C_MEM
    w_rec1 = np.asarray(w_rec1, np.float32) * C_MEM
    w_outs = np.asarray(w_out, np.float32) * C_MEM

    # FM layout: chunk m holds pooled rows p in {2m, 2m+1}, partition
    # (p%2)*32 + c  <->  reference feature index c*14 + p
    wi0 = np.zeros((64, 7, H0), np.float32)
    for m in range(7):
        for q in range(2):
            p = 2 * m + q
            for c in range(32):
                wi0[q * 32 + c, m, :] = w_in0[:, c * FH + p]
    d['WI0'] = wi0.astype(bf16)

    wr0 = np.zeros((96, 2, H0), np.float32)
    for j in range(2):
        wr0[:, j, :] = w_rec0[:, 96 * j:96 * (j + 1)].T
    d['WR0'] = wr0.astype(bf16)
    wi1 = np.zeros((96, 2, H1), np.float32)
    for j in range(2):
        wi1[:, j, :] = w_in1[:, 96 * j:96 * (j + 1)].T
    d['WI1'] = wi1.astype(bf16)
    d['WR1'] = np.ascontiguousarray(w_rec1.T).astype(bf16)
    d['WOUT'] = np.ascontiguousarray(w_outs.T).astype(bf16)

    fmb = np.zeros((64, 1), np.float32)
    for q in range(2):
        fmb[q * 32:(q + 1) * 32, 0] = beta5
    d['FMB'] = fmb
    return d


def _prep_images(images4):
    """[4,1,80,200] fp32 -> [44, 2, 4, 200] fp32, pre-scaled by C_MEM."""
    img = np.asarray(images4, np.float32)[:, 0] * C_MEM
    out = np.zeros((C1_WIN, 2, G, W_IMG), np.float32)
    out[:, 0] = img[:, 0:44].transpose(1, 0, 2)
    out[:38, 1] = img[:, 42:80].transpose(1, 0, 2)
    return out


# ------------------------------------------------------------- device build --

def build_module(T=T_FULL, debug_dump=False):
    from contextlib import ExitStack
    import concourse.bass as bass
    import concourse.mybir as mybir
    import concourse.tile as tile
    from concourse import bacc

    f32 = mybir.dt.float32
    bf = mybir.dt.bfloat16
    Alu = mybir.AluOpType
    Act = mybir.ActivationFunctionType

    nc = bacc.Bacc("TRN2", target_bir_lowering=False, debug=False, num_devices=1)

    img_d = nc.dram_tensor("img", (C1_WIN, 2, G, W_IMG), f32,
                           kind="ExternalInput").ap()
    wd = {}
    wd['L1'] = nc.dram_tensor("L1", (C1_WIN, 6, C1_COLS), bf,
                              kind="ExternalInput").ap()
    for l in range(2, 6):
        c = LCFG[l]
        wd[f'L{l}'] = nc.dram_tensor(f"L{l}", (c['pc'], c['k'], c['cols']),
                                     bf, kind="ExternalInput").ap()
    wd['WI0'] = nc.dram_tensor("WI0", (64, 7, H0), bf, kind="ExternalInput").ap()
    wd['WR0'] = nc.dram_tensor("WR0", (96, 2, H0), bf, kind="ExternalInput").ap()
    wd['WI1'] = nc.dram_tensor("WI1", (96, 2, H1), bf, kind="ExternalInput").ap()
    wd['WR1'] = nc.dram_tensor("WR1", (128, H1), bf, kind="ExternalInput").ap()
    wd['WOUT'] = nc.dram_tensor("WOUT", (128, ALPHABET), bf,
                                kind="ExternalInput").ap()
    wd['FMB'] = nc.dram_tensor("FMB", (64, 1), f32, kind="ExternalInput").ap()
    volts_d = nc.dram_tensor("volts", (ALPHABET, G, FW), f32,
                             kind="ExternalOutput").ap()
    dbg = {}
    if debug_dump:
        dbg['z0t'] = nc.dram_tensor("dbg_z0t", (C1_WIN, 2, G, W_IMG), f32,
                                    kind="ExternalOutput").ap()
        dbg['pool1'] = nc.dram_tensor("dbg_pool1", (C1_COLS, 2, G, 100), f32,
                                      kind="ExternalOutput").ap()
        dbg['fm'] = nc.dram_tensor("dbg_fm", (64, 7, G, FW), f32,
                                   kind="ExternalOutput").ap()
        dbg['i0'] = nc.dram_tensor("dbg_i0", (96, 2, NPIX), f32,
                                   kind="ExternalOutput").ap()
        dbg['z0'] = nc.dram_tensor("dbg_z0", (96, 2, NPIX), f32,
                                   kind="ExternalOutput").ap()
        dbg['v1'] = nc.dram_tensor("dbg_v1", (H1, NPIX), f32,
                                   kind="ExternalOutput").ap()

    with tile.TileContext(nc) as tc, ExitStack() as ctx:
        wpool = ctx.enter_context(tc.tile_pool(name="weights", bufs=1))
        spool = ctx.enter_context(tc.tile_pool(name="states", bufs=1))
        zpool = ctx.enter_context(tc.tile_pool(name="zenc", bufs=2))
        apool = ctx.enter_context(tc.tile_pool(name="acts", bufs=2))
        tpool = ctx.enter_context(tc.tile_pool(name="temps", bufs=2))
        ppool = ctx.enter_context(tc.tile_pool(name="ps", bufs=5, space="PSUM"))
        lpool = ctx.enter_context(tc.tile_pool(name="psl", bufs=3, space="PSUM"))

        wsb = {}
        for k in ['L1', 'L2', 'L3', 'L4', 'L5', 'WI0', 'WR0', 'WI1', 'WR1',
                  'WOUT', 'FMB']:
            shape = list(wd[k].shape)
            wsb[k] = wpool.tile(shape, wd[k].dtype, tag=f"w_{k}", name=f"w_{k}")
            nc.sync.dma_start(wsb[k][:], wd[k][:])

        v_enc = spool.tile([C1_WIN, 2, G, W_IMG], f32, tag="v_enc")
        img_sb = spool.tile([C1_WIN, 2, G, W_IMG], f32, tag="img_sb")
        nc.sync.dma_start(img_sb[:], img_d[:])

        v0 = spool.tile([96, 2, NPIX], f32, tag="v0")
        i0 = spool.tile([96, 2, NPIX], f32, tag="i0")
        b0 = spool.tile([96, 2, NPIX], f32, tag="b0")
        z0 = spool.tile([96, 2, NPIX], bf, tag="z0")
        v1 = spool.tile([H1, NPIX], f32, tag="v1")
        i1 = spool.tile([H1, NPIX], f32, tag="i1")
        b1 = spool.tile([H1, NPIX], f32, tag="b1")
        z1 = spool.tile([H1, NPIX], bf, tag="z1")
        vli = spool.tile([ALPHABET, NPIX], f32, tag="vli")
        ili = spool.tile([ALPHABET, NPIX], f32, tag="ili")
        vmax = spool.tile([ALPHABET, NPIX], f32, tag="vmax")

        nc.gpsimd.memset(v_enc[:], 0.0)
        for t_ in (v0, i0, v1, i1, vli, ili):
            nc.gpsimd.memset(t_[:], 0.0)
        nc.gpsimd.memset(z0[:], 0.0)
        nc.gpsimd.memset(z1[:], 0.0)
        nc.gpsimd.memset(b0[:], float(V_TH))
        nc.gpsimd.memset(b1[:], float(V_TH))
        nc.gpsimd.memset(vmax[:], -1e30)
        cad96 = spool.tile([96, 1], f32, tag="cad96")
        cad128 = spool.tile([128, 1], f32, tag="cad128")
        nc.gpsimd.memset(cad96[:], float(C_AD * V_TH))
        nc.gpsimd.memset(cad128[:], float(C_AD * V_TH))

        def emit_encoder(t):
            """LIF encoder step t -> spike tile (runs a step ahead)."""
            z_t = zpool.tile([C1_WIN, 2, G, W_IMG], bf, tag="z_t", name="z_t")
            nc.vector.scalar_tensor_tensor(
                out=v_enc[:], in0=v_enc[:], scalar=float(1.0 - C_MEM),
                in1=img_sb[:], op0=Alu.mult, op1=Alu.add)
            nc.vector.tensor_scalar(
                out=z_t[:], in0=v_enc[:], scalar1=float(V_TH), scalar2=None,
                op0=Alu.is_gt)
            nc.vector.scalar_tensor_tensor(
                out=v_enc[:], in0=v_enc[:], scalar=float(V_TH),
                in1=v_enc[:], op0=Alu.is_le, op1=Alu.mult)
            if debug_dump and t == 0:
                zf = tpool.tile([C1_WIN, 2, G, W_IMG], f32, tag="dbg_zf")
                nc.vector.tensor_copy(out=zf[:], in_=z_t[:])
                nc.sync.dma_start(dbg['z0t'][:], zf[:])
            return z_t

        def emit_conv_stages(t, z_t):
            """Conv stack for step t on spikes z_t; yields between groups."""

            # conv1 + pool -> pooled1 [126, 2, G, 100] bf16
            pooled1 = apool.tile([C1_COLS, 2, G, 100], bf, tag="pooled1",
                                 name="pooled1")
            for blk in range(2):
                for xh in range(2):
                    x0 = xh * 100
                    xw = 100 if xh == 0 else 98
                    pxw = xw // 2
                    pp = {}
                    for par in range(2):
                        ps = ppool.tile([C1_COLS, 512], f32, tag="ps",
                                        name="ps_c1")
                        psv = ps[:, :G * xw].rearrange("p (g x) -> p g x", g=G)
                        for kx in range(3):
                            nc.tensor.matmul(
                                psv,
                                wsb['L1'][:, par * 3 + kx, :],
                                z_t[:, blk, :, x0 + kx:x0 + kx + xw],
                                start=(kx == 0), stop=(kx == 2))
                        pe = tpool.tile([C1_COLS, G, pxw], f32,
                                        tag=f"p1_{par}", name=f"p1_{par}")
                        nc.vector.tensor_reduce(
                            pe[:],
                            psv.rearrange("p g (x two) -> p g x two", two=2),
                            axis=mybir.AxisListType.X, op=Alu.max,
                            opt_input=False)
                        pp[par] = pe
                    nc.vector.tensor_max(
                        out=pooled1[:, blk, :, x0 // 2:x0 // 2 + pxw],
                        in0=pp[0][:], in1=pp[1][:])
            if debug_dump and t == 0:
                pf = tpool.tile([C1_COLS, 2, G, 100], f32, tag="dbg_pf")
                nc.vector.memset(pf[:], 0.0)
                nc.vector.tensor_copy(out=pf[:, :, :, :99],
                                      in_=pooled1[:, :, :, :99])
                nc.sync.dma_start(dbg['pool1'][:], pf[:])

            # conv2 input windows [s, s+12) for s in S2B, from pooled1
            w2in = apool.tile([72, 4, G, 100], bf, tag="w2in", name="w2in")
            for p0 in range(32, 72, 32):     # last block pad rows 54..72
                nc.gpsimd.memset(w2in[p0:min(p0 + 32, 72), 3], 0.0)
            dq = [nc.sync, nc.gpsimd]
            nq = 0
            for b, s in enumerate(S2B):
                ylo, yhi = s, min(s + 12, 39)
                segs = []
                if ylo < 21:
                    segs.append((0, ylo, min(yhi, 21)))
                if yhi > 21:
                    segs.append((1, max(ylo, 21), yhi))
                for (sb, y0, y1) in segs:
                    srow = (y0 - (0 if sb == 0 else 21)) * 6
                    drow = (y0 - ylo) * 6
                    nr = (y1 - y0) * 6
                    dq[nq % 2].dma_start(
                        w2in[drow:drow + nr, b, :, :99],
                        pooled1[srow:srow + nr, sb, :, :99])
                    nq += 1

            yield None
            # conv2 (4 non-overlapping blocks) -> bf16 staging
            s2 = []
            for b in range(4):
                ps = ppool.tile([120, 512], f32, tag="ps", name="ps_l2")
                psv = ps[:, :G * 97].rearrange("p (g x) -> p g x", g=G)
                for kx in range(3):
                    nc.tensor.matmul(
                        psv, wsb['L2'][:, kx, :],
                        w2in[:, b, :, kx:kx + 97],
                        start=(kx == 0), stop=(kx == 2))
                st2 = apool.tile([120, G, 97], bf, tag=f"st2_{b}",
                                 name=f"st2_{b}")
                nc.scalar.copy(out=st2[:], in_=psv)
                s2.append(st2)

            # conv3 input windows [6b2, 6b2+10) scattered from s2
            win3 = apool.tile([120, 6, G, 97], bf, tag="win3", name="win3")
            for p0 in range(64, 120, 32):    # last window pad rows 84..120
                nc.gpsimd.memset(win3[p0:min(p0 + 32, 120), 5], 0.0)
            for b2 in range(6):
                ylo = 6 * b2
                yhi = min(ylo + 10, 37)
                y = ylo
                while y < yhi:
                    pb = y // 10
                    y1 = min(yhi, (pb + 1) * 10)
                    srow = (y - pb * 10) * 12
                    drow = (y - ylo) * 12
                    nr = (y1 - y) * 12
                    dq[nq % 2].dma_start(
                        win3[drow:drow + nr, b2, :, :],
                        s2[pb][srow:srow + nr, :, :])
                    nq += 1
                    y = y1

            yield None
            # conv3 -> bf16 staging (6 blocks, yob=6)
            s3 = []
            for b in range(6):
                ps = ppool.tile([96, 512], f32, tag="ps", name="ps_l3")
                psv = ps[:, :G * 93].rearrange("p (g x) -> p g x", g=G)
                for kx in range(5):
                    nc.tensor.matmul(
                        psv, wsb['L3'][:, kx, :],
                        win3[:, b, :, kx:kx + 93],
                        start=(kx == 0), stop=(kx == 4))
                st = apool.tile([96, G, 93], bf, tag=f"st3_{b}",
                                name=f"st3_{b}")
                nc.scalar.copy(out=st[:], in_=psv)
                s3.append(st)

            # conv4 input windows [4m, 4m+8), from s3 (blocks of 6)
            w4in = apool.tile([128, 8, G, 93], bf, tag="w4in", name="w4in")
            for p0 in range(64, 128, 32):    # window 7 pad rows 80..128
                nc.gpsimd.memset(w4in[p0:p0 + 32, 7], 0.0)
            for m, w in enumerate(W4_STARTS):
                y = w
                yhi = min(w + 8, 33)
                while y < yhi:
                    pb = y // 6
                    y1 = min(yhi, (pb + 1) * 6)
                    srow = (y - pb * 6) * 16
                    drow = (y - w) * 16
                    nr = (y1 - y) * 16
                    dq[nq % 2].dma_start(
                        w4in[drow:drow + nr, m, :, :],
                        s3[pb][srow:srow + nr, :, :])
                    nq += 1
                    y = y1

            yield None
            # conv4: 8 blocks of 5 rows at stride 4 (= conv5 E windows +1).
            # E windows are direct casts; O windows [4m+1,4m+6) are gathered
            # from the E casts by small SBUF DMAs.
            win5 = apool.tile([120, 14, G, 91], bf, tag="win5", name="win5")
            e7 = apool.tile([120, G, 91], bf, tag="e7", name="e7")
            for m in range(8):
                ps = ppool.tile([120, 512], f32, tag="ps", name="ps_l4")
                psv = ps[:, :G * 91].rearrange("p (g x) -> p g x", g=G)
                for kx in range(3):
                    nc.tensor.matmul(
                        psv, wsb['L4'][:, kx, :],
                        w4in[:, m, :, kx:kx + 91],
                        start=(kx == 0), stop=(kx == 2))
                if m < 7:
                    nc.scalar.copy(out=win5[:, m], in_=psv)
                else:
                    nc.scalar.copy(out=e7[:], in_=psv)
                if m >= 1:
                    # O window m-1: rows 1..4 of E block m-1 + row 1 of E m
                    mm_ = m - 1
                    dq[nq % 2].dma_start(
                        win5[0:96, 7 + mm_], win5[24:120, mm_])
                    dq[nq % 2].dma_start(
                        win5[96:120, 7 + mm_],
                        (win5[24:48, m] if m < 7 else e7[24:48]))
                    nq += 1

            # conv5: E_m then O_m interleaved; pool-y pair m fires right
            # after O_m so fm chunks complete incrementally. The LSNN i0
            # matmuls for this step interleave chunk-by-chunk to keep PE
            # duty high across the conv->LSNN transition.
            ps_i0 = [lpool.tile([96, NPIX], f32, tag="psl", name="ps_i0a"),
                     lpool.tile([96, NPIX], f32, tag="psl", name="ps_i0b")]
            fms = []
            for m in range(7):
                pxp = {}
                for par in range(2):
                    bi = par * 7 + m
                    ps = ppool.tile([64, 512], f32, tag="ps", name="ps_l5")
                    psv = ps[:, :G * 89].rearrange("p (g x) -> p g x", g=G)
                    for kx in range(3):
                        nc.tensor.matmul(
                            psv, wsb['L5'][:, kx, :],
                            win5[:, bi, :, kx:kx + 89],
                            start=(kx == 0), stop=(kx == 2))
                    px = tpool.tile([64, G, FW], f32, tag=f"p5x_{par}",
                                    name=f"p5x_{par}")
                    nc.vector.tensor_reduce(
                        px[:], psv[:, :, :2 * FW].rearrange(
                            "p g (x two) -> p g x two", two=2),
                        axis=mybir.AxisListType.X, op=Alu.max, opt_input=False)
                    pxp[par] = px
                fmst = tpool.tile([64, G, FW], f32, tag="fmst", name="fmst")
                for q in range(2):
                    nc.vector.tensor_max(
                        out=fmst[q * 32:q * 32 + 32],
                        in0=pxp[0][q * 32:q * 32 + 32],
                        in1=pxp[1][q * 32:q * 32 + 32])
                fmc = apool.tile([64, G, FW], bf, tag=f"fm_{m}",
                                 name=f"fm_{m}")
                nc.scalar.activation(
                    out=fmc[:], in_=fmst[:], func=Act.Identity,
                    bias=wsb['FMB'][:, 0:1], scale=1.0)
                fms.append(fmc)
                for mh in range(2):
                    nc.tensor.matmul(
                        ps_i0[mh][:], wsb['WI0'][:, m, 96 * mh:96 * (mh + 1)],
                        fmc.rearrange("p g x -> p (g x)"),
                        start=(m == 0), stop=False)
            for mh in range(2):
                for j in range(2):
                    nc.tensor.matmul(
                        ps_i0[mh][:], wsb['WR0'][:, j, 96 * mh:96 * (mh + 1)],
                        z0[:, j], start=False, stop=(j == 1))
            if debug_dump and t == 0:
                for m in range(7):
                    ff = tpool.tile([64, G, FW], f32, tag="dbg_ff")
                    nc.vector.tensor_copy(out=ff[:], in_=fms[m][:])
                    nc.sync.dma_start(dbg['fm'][:, m], ff[:])
            yield (fms, ps_i0)

        def emit_lsnn_stages(t, ps_i0):
            """LSNN + LI for step t, staged for interleaving with conv.
            ps_i0 comes pre-accumulated from the conv5 stage."""
            yield None

            # z-spike on the shortest possible DVE chain, state updates after
            vdec0 = tpool.tile([96, 2, NPIX], f32, tag="vdec0", name="vdec0")
            bdec0 = tpool.tile([96, 2, NPIX], f32, tag="bdec0", name="bdec0")
            zn0 = tpool.tile([96, 2, NPIX], f32, tag="zn0", name="zn0")
            nc.scalar.activation(
                out=bdec0[:], in_=b0[:], func=Act.Identity,
                bias=cad96[:, 0:1], scale=float(1.0 - C_AD))
            nc.vector.scalar_tensor_tensor(
                out=vdec0[:], in0=v0[:], scalar=float(1.0 - C_MEM), in1=i0[:],
                op0=Alu.mult, op1=Alu.add)
            nc.vector.tensor_tensor(
                out=z0[:], in0=vdec0[:], in1=bdec0[:], op=Alu.is_gt)
            yield None

            ps_i1 = lpool.tile([H1, NPIX], f32, tag="psl", name="ps_i1")
            for j in range(2):
                nc.tensor.matmul(
                    ps_i1[:], wsb['WI1'][:, j, :], z0[:, j],
                    start=(j == 0), stop=False)
            nc.tensor.matmul(ps_i1[:], wsb['WR1'][:], z1[:],
                             start=False, stop=True)

            # layer-1 z on the short chain
            vdec1 = tpool.tile([H1, NPIX], f32, tag="vdec1", name="vdec1")
            bdec1 = tpool.tile([H1, NPIX], f32, tag="bdec1", name="bdec1")
            zn1 = tpool.tile([H1, NPIX], f32, tag="zn1", name="zn1")
            nc.scalar.activation(
                out=bdec1[:], in_=b1[:], func=Act.Identity,
                bias=cad128[:, 0:1], scale=float(1.0 - C_AD))
            nc.vector.scalar_tensor_tensor(
                out=vdec1[:], in0=v1[:], scalar=float(1.0 - C_MEM), in1=i1[:],
                op0=Alu.mult, op1=Alu.add)
            nc.vector.tensor_tensor(
                out=z1[:], in0=vdec1[:], in1=bdec1[:], op=Alu.is_gt)

            # layer-0 state updates (off the z critical path)
            nc.vector.tensor_tensor(
                out=zn0[:], in0=vdec0[:], in1=bdec0[:], op=Alu.is_le)
            nc.vector.tensor_mul(out=v0[:], in0=zn0[:], in1=vdec0[:])
            nc.vector.scalar_tensor_tensor(
                out=b0[:], in0=z0[:], scalar=float(C_BETA), in1=bdec0[:],
                op0=Alu.mult, op1=Alu.add)
            for m in range(2):
                nc.vector.scalar_tensor_tensor(
                    out=i0[:, m], in0=i0[:, m], scalar=float(1.0 - C_SYN),
                    in1=ps_i0[m][:], op0=Alu.mult, op1=Alu.add)
            if debug_dump and t == T - 1:
                z0f = tpool.tile([96, 2, NPIX], f32, tag="dbg_z0f")
                nc.vector.tensor_copy(out=z0f[:], in_=z0[:])
                nc.sync.dma_start(dbg['z0'][:], z0f[:])
                nc.sync.dma_start(dbg['i0'][:], i0[:])
            yield None

            ps_li = lpool.tile([ALPHABET, NPIX], f32, tag="psl", name="ps_li")
            nc.tensor.matmul(ps_li[:], wsb['WOUT'][:], z1[:],
                             start=True, stop=True)

            # layer-1 state updates (off the z critical path)
            nc.vector.tensor_tensor(
                out=zn1[:], in0=vdec1[:], in1=bdec1[:], op=Alu.is_le)
            nc.vector.tensor_mul(out=v1[:], in0=zn1[:], in1=vdec1[:])
            nc.vector.scalar_tensor_tensor(
                out=b1[:], in0=z1[:], scalar=float(C_BETA), in1=bdec1[:],
                op0=Alu.mult, op1=Alu.add)
            nc.vector.scalar_tensor_tensor(
                out=i1[:], in0=i1[:], scalar=float(1.0 - C_SYN),
                in1=ps_i1[:], op0=Alu.mult, op1=Alu.add)
            if debug_dump and t == T - 1:
                nc.sync.dma_start(dbg['v1'][:], v1[:])
            nc.vector.scalar_tensor_tensor(
                out=vli[:], in0=vli[:], scalar=float(1.0 - C_MEM), in1=ili[:],
                op0=Alu.mult, op1=Alu.add)
            nc.vector.tensor_max(out=vmax[:], in0=vmax[:], in1=vli[:])
            nc.vector.scalar_tensor_tensor(
                out=ili[:], in0=ili[:], scalar=float(1.0 - C_SYN),
                in1=ps_li[:], op0=Alu.mult, op1=Alu.add)
            yield None

        # conv runs one step ahead of the LSNN; the encoder runs one step
        # ahead of the conv; LSNN matmul groups are interleaved between conv
        # groups to keep PE duty high
        zs = {0: emit_encoder(0)}
        fm_prev = None
        for _r in emit_conv_stages(0, zs[0]):
            if _r is not None:
                fm_prev = _r
        if T > 1:
            zs[1] = emit_encoder(1)
        for t in range(T):
            st = emit_lsnn_stages(t, fm_prev[1])
            next(st)                                   # i0 matmuls
            parts = (emit_conv_stages(t + 1, zs[t + 1])
                     if t + 1 < T else None)
            if parts is not None:
                next(parts)                            # conv1 + w2in
            next(st)                                   # el0-z
            if parts is not None:
                next(parts)                            # conv2
            next(st)                                   # i1mm + el1-z + states0
            if parts is not None:
                next(parts)                            # conv3 + w4in
            next(st)                                   # limm + states1 + li
            if t + 2 < T:
                zs[t + 2] = emit_encoder(t + 2)
            fm_next = next(parts) if parts is not None else None
            for _ in st:
                pass
            fm_prev = fm_next
            zs.pop(t, None)

        nc.sync.dma_start(volts_d[:], vmax.rearrange("p (g x) -> p g x", g=G))

    nc.compile()
    return nc


_NC_CACHE = {}


def _get_module(T=T_FULL, debug_dump=False):
    key = (T, debug_dump)
    if key not in _NC_CACHE:
        _NC_CACHE[key] = build_module(T, debug_dump)
    return _NC_CACHE[key]


# ------------------------------------------------------------------ kernel --

def kernel(images_batch, fe_params, w_in0, w_rec0, w_in1, w_rec1, w_out,
           T=T_FULL, debug_dump=False, trace=False):
    from concourse.bass_utils import run_bass_kernel_spmd

    images_batch = np.asarray(images_batch, np.float32)
    wdict = _prep_weights(fe_params, w_in0, w_rec0, w_in1, w_rec1, w_out)
    nc = _get_module(T, debug_dump)

    in_maps = []
    for c in range(N_CORES):
        m = {'img': _prep_images(images_batch[G * c:G * (c + 1)])}
        for k, v in wdict.items():
            m[k] = v
        in_maps.append(m)

    res = run_bass_kernel_spmd(nc, in_maps, core_ids=list(range(N_CORES)),
                               trace=trace)

    voltages = np.zeros((B_FULL, FW, ALPHABET), np.float32)
    for c in range(N_CORES):
        v = res.results[c]['volts']          # [37, G, FW]
        voltages[G * c:G * (c + 1)] = v.transpose(1, 2, 0)
    voltages_length = np.full((B_FULL,), FW, np.int32)
    if debug_dump or trace:
        kernel._last_res = res
    return voltages, voltages_length
